# revision 21
# baseline (speedup 1.0000x reference)
"""DeepseekV3 MLA prefill attention on 8 trn2 NeuronCores.

Strategy (single SPMD program, per-core differences live in the input data):
  Phase A: token-split A-projection, computed feature-major
           (qkv^T = W_a^T @ h^T), fused RMSNorm (partition-dim reduce via
           ones-matmul), RoPE on k_pe. gamma and the 1/sqrt(d) score scale
           are folded into the weights on the host; RoPE de-interleave is
           folded into weight column order on the host.
  AG1:     AllGather of normed latents (bf16, feature-major).
  Phase B: per-core head projections Q^T, K^T (feature-major) and V
           (token-major), heads 2c and 2c+1 on core c.
  Phase C: causal attention, S^T = K^T-tiles x Q^T-chunks, exp without
           max-subtraction (scores are O(+-8) by construction), softmax
           denominator via ones-matmul, PV accumulated feature-major,
           block-causal skipping of fully-masked tiles.
  AG2:     AllGather of attention outputs (bf16, feature-major).
  Phase E: column-split o_proj (core c computes output cols 896c..896(c+1)),
           host concatenates.
"""

import numpy as np
import ml_dtypes

T = 2048
H = 7168
NH = 16
D_NOPE = 128
D_ROPE = 64
D_V = 128
D_QK = 192
QLR = 1536
KVLR = 512
THETA = 10000.0
EPS = 1e-6
NCORES = 8
TSH = T // NCORES          # 256 tokens per core
HPC = NH // NCORES         # 2 heads per core
WO_COLS = H // NCORES      # 896 output cols per core
AGF = QLR + KVLR + D_ROPE  # 2112 gathered feature rows
NEG = -30000.0             # mask add, enough to zero bf16/f32 exp

BF16 = ml_dtypes.bfloat16

_CACHE = {}


class _Done(Exception):
    pass


def _build(upto='E'):
    import concourse.bass as bass
    import concourse.mybir as mybir
    import concourse.bacc as bacc
    import concourse.tile as tile

    dt = mybir.dt
    AF = mybir.ActivationFunctionType

    nc = bacc.Bacc(None, target_bir_lowering=False)

    # ---- per-core external inputs -------------------------------------
    hT = nc.declare_dram_parameter("hT", [H, TSH], dt.bfloat16, isOutput=False)
    wa = nc.declare_dram_parameter("wa", [17 * 56 * 128, 128], dt.bfloat16, isOutput=False)
    wqb = nc.declare_dram_parameter("wqb", [QLR, HPC * D_QK], dt.bfloat16, isOutput=False)
    wkvb = nc.declare_dram_parameter("wkvb", [KVLR, HPC * 256], dt.bfloat16, isOutput=False)
    wo = nc.declare_dram_parameter("wo", [NH * D_V, WO_COLS], dt.bfloat16, isOutput=False)
    cs_sh = nc.declare_dram_parameter("cs_sh", [64, TSH], dt.float32, isOutput=False)
    cs_full = nc.declare_dram_parameter("cs_full", [64, T], dt.float32, isOutput=False)
    masks = nc.declare_dram_parameter("masks", [4 * 128, 512], dt.float32, isOutput=False)
    out = nc.declare_dram_parameter("out", [WO_COLS, T], dt.float32, isOutput=True)

    rg = [list(range(NCORES))]

    _build_body(nc, mybir, upto, hT, wa, wqb, wkvb, wo, cs_sh, cs_full,
                masks, out)
    nc.compile()
    return nc


def _build_body(nc, mybir, upto, hT, wa, wqb, wkvb, wo, cs_sh, cs_full,
                masks, out):
    import concourse.tile as tile
    dt = mybir.dt
    AF = mybir.ActivationFunctionType
    rg = [list(range(NCORES))]

    with tile.TileContext(nc) as tc:
        import contextlib

        top = contextlib.ExitStack()
        with top:
            const = top.enter_context(tc.tile_pool(name="const", bufs=1))
            wpool = top.enter_context(tc.tile_pool(name="wpool", bufs=1))
            dram = top.enter_context(tc.tile_pool(name="dram", bufs=1, space="DRAM"))

            ones_b = const.tile([128, 1], dt.bfloat16, tag="ones_b", name="ones_b")
            nc.vector.memset(ones_b[:], 1.0)
            ones_f = const.tile([1, 128], dt.float32, tag="ones_f", name="ones_f")
            nc.vector.memset(ones_f[:], 1.0)
            # tiles allocated up front; DMAs for B/C-phase constants are
            # emitted at point of need so phase A's h/wa stream goes first
            mask_sb = const.tile([128, 4, 512], dt.float32, tag="mask", name="mask")
            # cos/sin replicated to every 32-partition block so DVE ops
            # stay partition-aligned for both heads (loads emitted at m==11)
            csc_r = const.tile([128, T], dt.float32, tag="csc_r", name="csc_r")
            csn_r = const.tile([128, T], dt.float32, tag="csn_r", name="csn_r")
            csc_s = const.tile([32, TSH], dt.float32, tag="csc_s", name="csc_s")
            nc.sync.dma_start(csc_s[:], cs_sh[0:32, :])
            csn_s = const.tile([32, TSH], dt.float32, tag="csn_s", name="csn_s")
            nc.sync.dma_start(csn_s[:], cs_sh[32:64, :])

            # resident weights for phases B and E (loads emitted later)
            wqb_t = wpool.tile([128, 12, HPC * D_QK], dt.bfloat16, tag="wqb", name="wqb")
            wkvb_t = wpool.tile([128, 4, HPC * 256], dt.bfloat16, tag="wkvb", name="wkvb")
            wo_t = wpool.tile([128, 16, WO_COLS], dt.bfloat16, tag="wo", name="wo")

            # collective buffers (AG1 split: kv+k_pe gathered early, q late;
            # AG2 split per head so it overlaps the other head's attention)
            ag1a_in = dram.tile([576, TSH], dt.bfloat16, tag="ag1ai", name="ag1ai")
            ag1a_out = dram.tile([NCORES * 576, TSH], dt.bfloat16, tag="ag1ao", name="ag1ao", addr_space="Shared")
            ag1b_in = dram.tile([QLR, TSH], dt.bfloat16, tag="ag1bi", name="ag1bi")
            ag1b_out = dram.tile([NCORES * QLR, TSH], dt.bfloat16, tag="ag1bo", name="ag1bo", addr_space="Shared")
            # AG2 split per 512-token chunk: chunk j's gather overlaps
            # attention on chunk j+1 and o_proj on chunk j-1
            ag2_in = [dram.tile([2 * D_V, 512], dt.bfloat16, tag=f"ag2i{j}",
                                name=f"ag2i{j}") for j in range(4)]
            ag2_out = [dram.tile([NCORES * 2 * D_V, 512], dt.bfloat16,
                                 tag=f"ag2o{j}", name=f"ag2o{j}",
                                 addr_space="Shared") for j in range(4)]

            # ============================================================
            # Phase A: qkv^T = Wa^T @ h^T   [2112, 256] feature-major
            # ============================================================
            with contextlib.ExitStack() as pa:
                h_pool = pa.enter_context(tc.tile_pool(name="h", bufs=1))
                wa_pool = pa.enter_context(tc.tile_pool(name="wa", bufs=6))
                qkv_pool = pa.enter_context(tc.tile_pool(name="qkv", bufs=1))
                x2_pool = pa.enter_context(tc.tile_pool(name="x2", bufs=3))
                agt_pool = pa.enter_context(tc.tile_pool(name="agt", bufs=3))
                ps_a = pa.enter_context(tc.tile_pool(name="ps_a", bufs=3, space="PSUM"))
                ps_ss = pa.enter_context(tc.tile_pool(name="ps_ss", bufs=1, space="PSUM"))
                ps_bc = pa.enter_context(tc.tile_pool(name="ps_bc", bufs=1, space="PSUM"))

                h_all = h_pool.tile([128, 56, TSH], dt.bfloat16, tag="h_all", name="h_all")
                for a0 in range(0, 56, 7):
                    nc.sync.dma_start(
                        h_all[:, a0:a0 + 7, :],
                        hT[a0 * 128:(a0 + 7) * 128, :].rearrange(
                            "(a p) t -> p a t", p=128))

                qkv = [
                    qkv_pool.tile([128, TSH], dt.float32, tag=f"qkv{m}", name=f"qkv{m}")
                    for m in range(16)
                ]
                kp_raw = qkv_pool.tile([64, TSH], dt.float32, tag="kp_raw", name="kp_raw")
                kp2 = qkv_pool.tile([32, TSH], dt.float32, tag="kp2", name="kp2")

                ss_q = ps_ss.tile([1, TSH], dt.float32, tag="ssq", name="ssq")
                ss_kv = ps_ss.tile([1, TSH], dt.float32, tag="sskv", name="sskv")

                def rstd_bcast(ss, d, name):
                    ms = x2_pool.tile([1, TSH], dt.float32, tag="ms", name="ms")
                    nc.scalar.activation(ms[:], ss[:], AF.Copy, bias=EPS, scale=1.0 / d)
                    inv = x2_pool.tile([1, TSH], dt.float32, tag="inv", name="inv")
                    nc.vector.reciprocal_approx_fast(inv[:], ms[:])
                    rstd = x2_pool.tile([1, TSH], dt.float32, tag="rstd", name="rstd")
                    nc.scalar.activation(rstd[:], inv[:], AF.Sqrt)
                    pb = ps_bc.tile([128, TSH], dt.float32, tag=f"bc{name}", name=f"bc{name}")
                    nc.tensor.matmul(pb[:], ones_f[:], rstd[:], start=True, stop=True)
                    return pb

                for m in list(range(12)) + [12, 13, 14, 15, 16]:
                    mp = 64 if m == 16 else 128
                    psum = ps_a.tile([128, TSH], dt.float32, tag="pa", name="pa")
                    for kc in range(7):
                        chunk = wa_pool.tile([128, 8, 128], dt.bfloat16, tag="wa_c", name="wa_c")
                        r0 = (m * 56 + kc * 8) * 128
                        nc.sync.dma_start(
                            chunk[:],
                            wa[r0:r0 + 1024, :].rearrange("(p a) f -> p a f", a=8),
                        )
                        for k8 in range(8):
                            k = kc * 8 + k8
                            nc.tensor.matmul(
                                psum[:mp, :],
                                chunk[:, k8, :mp],
                                h_all[:, k, :],
                                start=(k == 0),
                                stop=(k == 55),
                                skip_group_check=True,
                            )
                    # evict to f32 SBUF
                    if m < 16:
                        nc.scalar.copy(qkv[m][:], psum[:])
                    else:
                        nc.scalar.copy(kp_raw[:], psum[:64, :])
                        # move the x2 half to base partition 0 for the DVE ops
                        nc.sync.dma_start(kp2[:], kp_raw[32:64, :])
                    if m < 16:
                        # squared tile for the RMS partition-sum
                        x2 = x2_pool.tile([128, TSH], dt.bfloat16, tag="x2", name="x2")
                        nc.vector.tensor_mul(x2[:], qkv[m][:], qkv[m][:])
                        ss = ss_q if m < 12 else ss_kv
                        first = (m == 0) or (m == 12)
                        last = (m == 11) or (m == 15)
                        nc.tensor.matmul(
                            ss[:], ones_b[:], x2[:], start=first, stop=last,
                            skip_group_check=True,
                        )
                    if m == 11:
                        # q group done: norm q, launch AG1b early so the
                        # gather overlaps the kv-group matmuls
                        bc_q = rstd_bcast(ss_q, QLR, "q")
                        for mm in range(12):
                            agt = agt_pool.tile([128, TSH], dt.bfloat16, tag="agt", name="agt")
                            nc.vector.tensor_mul(agt[:], qkv[mm][:], bc_q[:])
                            nc.sync.dma_start(
                                ag1b_in[mm * 128:(mm + 1) * 128, :], agt[:])
                        # B-phase weights: load now, overlapping the gather
                        for a0 in range(0, 12, 4):
                            nc.sync.dma_start(
                                wqb_t[:, a0:a0 + 4, :],
                                wqb[a0 * 128:(a0 + 4) * 128, :].rearrange(
                                    "(a p) f -> p a f", p=128))
                        nc.sync.dma_start(
                            wkvb_t[:], wkvb.rearrange("(a p) f -> p a f", p=128))
                        for i in range(4):
                            nc.sync.dma_start(
                                mask_sb[:, i, :], masks[i * 128:(i + 1) * 128, :])
                        for b4 in range(4):
                            nc.sync.dma_start(
                                csc_r[b4 * 32:(b4 + 1) * 32, :], cs_full[0:32, :])
                            nc.sync.dma_start(
                                csn_r[b4 * 32:(b4 + 1) * 32, :], cs_full[32:64, :])
                    if m == 16:
                        # kv group + k_pe done: norm kv, rope k_pe, launch AG1a
                        bc_kv = rstd_bcast(ss_kv, KVLR, "kv")
                        for mm in range(12, 16):
                            agt = agt_pool.tile([128, TSH], dt.bfloat16, tag="agt", name="agt")
                            nc.vector.tensor_mul(agt[:], qkv[mm][:], bc_kv[:])
                            nc.sync.dma_start(
                                ag1a_in[(mm - 12) * 128:(mm - 11) * 128, :], agt[:])
                        kr1 = agt_pool.tile([32, TSH], dt.bfloat16, tag="kr1", name="kr1")
                        kr2 = agt_pool.tile([32, TSH], dt.bfloat16, tag="kr2", name="kr2")
                        t1 = x2_pool.tile([32, TSH], dt.bfloat16, tag="t1", name="t1")
                        t2 = x2_pool.tile([32, TSH], dt.bfloat16, tag="t2", name="t2")
                        nc.vector.tensor_mul(t1[:], kp_raw[0:32, :], csc_s[:])
                        nc.vector.tensor_mul(t2[:], kp2[:], csn_s[:])
                        nc.vector.tensor_sub(kr1[:], t1[:], t2[:])
                        t3 = x2_pool.tile([32, TSH], dt.bfloat16, tag="t1", name="t1")
                        t4 = x2_pool.tile([32, TSH], dt.bfloat16, tag="t2", name="t2")
                        nc.vector.tensor_mul(t3[:], kp_raw[0:32, :], csn_s[:])
                        nc.vector.tensor_mul(t4[:], kp2[:], csc_s[:])
                        nc.vector.tensor_add(kr2[:], t3[:], t4[:])
                        nc.sync.dma_start(ag1a_in[512:544, :], kr1[:])
                        nc.sync.dma_start(ag1a_in[544:576, :], kr2[:])
                        nc.gpsimd.collective_compute(
                            "AllGather", mybir.AluOpType.bypass,
                            replica_groups=rg,
                            ins=[ag1a_in.opt()], outs=[ag1a_out.opt()])
                        nc.gpsimd.collective_compute(
                            "AllGather", mybir.AluOpType.bypass,
                            replica_groups=rg,
                            ins=[ag1b_in.opt()], outs=[ag1b_out.opt()])

            if upto == 'A':
                return

            # ============================================================
            # Phase B: Q^T, K^T (feature-major) and V (token-major)
            # ============================================================
            bpools = contextlib.ExitStack()
            with bpools:
                act = bpools.enter_context(tc.tile_pool(name="act", bufs=1))
                agq_pool = bpools.enter_context(tc.tile_pool(name="agq", bufs=6))
                agkv_pool = bpools.enter_context(tc.tile_pool(name="agkv", bufs=2))
                pbp = contextlib.ExitStack()
                ps_kv = pbp.enter_context(tc.tile_pool(name="ps_kv", bufs=2, space="PSUM"))

                qt_n = [act.tile([128, T], dt.bfloat16, tag=f"qtn{h}", name=f"qtn{h}") for h in range(HPC)]
                # merged rope layout: partitions [64h, 64h+32) = head h x1,
                # [64h+32, 64h+64) = head h x2
                qrw2 = act.tile([128, T], dt.bfloat16, tag="qrw2", name="qrw2")
                qt_r2 = act.tile([128, T], dt.bfloat16, tag="qtr2", name="qtr2")
                kt_n = [act.tile([128, T], dt.bfloat16, tag=f"ktn{h}", name=f"ktn{h}") for h in range(HPC)]
                # k_pe replicated on both 64-partition halves so each
                # head's rope matmul is partition-aligned with qt_r2
                kpe_t = act.tile([128, T], dt.bfloat16, tag="kpet", name="kpet")
                # v2_t[ti]: both heads' V for token tile ti, cols h*128..
                v2_t = [act.tile([128, 2 * D_V], dt.bfloat16, tag=f"v{i}", name=f"v{i}")
                        for i in range(16)]

                # KV/V first: AG1a is the first (small) collective, so
                # K/V work fills the window while the big q gather runs
                for rp in range(4):
                    tsl = slice(rp * 512, (rp + 1) * 512)
                    # --- KV path ---
                    akv = [agkv_pool.tile([128, 512], dt.bfloat16, tag=f"akv{kk}", name=f"akv{kk}")
                           for kk in range(4)]
                    for kk in range(4):
                        for s in range(2):
                            r = 2 * rp + s
                            nc.sync.dma_start(
                                akv[kk][:, s * TSH:(s + 1) * TSH],
                                ag1a_out[r * 576 + kk * 128: r * 576 + (kk + 1) * 128, :])
                    for h in range(HPC):
                        pk = ps_kv.tile([128, 512], dt.float32, tag="pkv", name="pkv")
                        for kk in range(4):
                            nc.tensor.matmul(
                                pk[:], wkvb_t[:, kk, h * 128:(h + 1) * 128], akv[kk][:],
                                start=(kk == 0), stop=(kk == 3),
                                skip_group_check=True)
                        nc.scalar.copy(kt_n[h][:, tsl], pk[:])
                    # --- V for both heads at once (wkvb cols 256..512) ---
                    for s4 in range(4):
                        ti = 4 * rp + s4
                        pv = ps_kv.tile([128, 2 * D_V], dt.float32, tag="pv_b", name="pv_b")
                        for kk in range(4):
                            nc.tensor.matmul(
                                pv[:],
                                akv[kk][:, s4 * 128:(s4 + 1) * 128],
                                wkvb_t[:, kk, 256:512],
                                start=(kk == 0), stop=(kk == 3),
                                skip_group_check=True)
                        nc.scalar.copy(v2_t[ti][:], pv[:])
                    # --- shared roped k_pe (both halves) ---
                    for s in range(2):
                        r = 2 * rp + s
                        for hh in range(2):
                            nc.sync.dma_start(
                                kpe_t[64 * hh:64 * hh + 64, r * TSH:(r + 1) * TSH],
                                ag1a_out[r * 576 + 512: r * 576 + 576, :])

                pbp.close()
                pbp = contextlib.ExitStack()
                ps_q = pbp.enter_context(tc.tile_pool(name="ps_q", bufs=6, space="PSUM"))

                for rp in range(4):
                    tsl = slice(rp * 512, (rp + 1) * 512)
                    # --- Q path (two ranks per 512-wide chunk) ---
                    pn0 = ps_q.tile([128, 512], dt.float32, tag="pq", name="pq")
                    pn1 = ps_q.tile([128, 512], dt.float32, tag="pq", name="pq")
                    pr2 = ps_q.tile([128, 512], dt.float32, tag="pq", name="pq")
                    for kq in range(12):
                        aq = agq_pool.tile([128, 512], dt.bfloat16, tag="aq", name="aq")
                        for s in range(2):
                            r = 2 * rp + s
                            nc.sync.dma_start(
                                aq[:, s * TSH:(s + 1) * TSH],
                                ag1b_out[r * QLR + kq * 128: r * QLR + (kq + 1) * 128, :])
                        nc.tensor.matmul(
                            pn0[:], wqb_t[:, kq, 0:128], aq[:],
                            start=(kq == 0), stop=(kq == 11),
                            skip_group_check=True)
                        nc.tensor.matmul(
                            pn1[:], wqb_t[:, kq, 128:256], aq[:],
                            start=(kq == 0), stop=(kq == 11),
                            skip_group_check=True)
                        nc.tensor.matmul(
                            pr2[:], wqb_t[:, kq, 256:384], aq[:],
                            start=(kq == 0), stop=(kq == 11),
                            skip_group_check=True)
                    nc.scalar.copy(qt_n[0][:, tsl], pn0[:])
                    nc.scalar.copy(qt_n[1][:, tsl], pn1[:])
                    nc.scalar.copy(qrw2[:, tsl], pr2[:])
                    # --- RoPE on q for this 512-token chunk; per head all
                    # DVE ops live on partitions [64h, 64h+32) ---
                    for h in range(HPC):
                        p0 = 64 * h
                        x2c = agq_pool.tile([128, 512], dt.bfloat16, tag="x2c", name="x2c")
                        nc.sync.dma_start(
                            x2c[p0:p0 + 32, :], qrw2[p0 + 32:p0 + 64, tsl])
                        x1 = qrw2[p0:p0 + 32, tsl]
                        x2 = x2c[p0:p0 + 32, :]
                        cs_ = csc_r[p0:p0 + 32, tsl]
                        sn_ = csn_r[p0:p0 + 32, tsl]
                        ta = agq_pool.tile([128, 512], dt.bfloat16, tag="qt1", name="qt1")
                        tb = agq_pool.tile([128, 512], dt.bfloat16, tag="qt2", name="qt2")
                        nc.vector.tensor_mul(ta[p0:p0 + 32, :], x1, cs_)
                        nc.vector.tensor_mul(tb[p0:p0 + 32, :], x2, sn_)
                        nc.vector.tensor_sub(
                            qt_r2[p0:p0 + 32, tsl], ta[p0:p0 + 32, :], tb[p0:p0 + 32, :])
                        tg = agq_pool.tile([128, 512], dt.bfloat16, tag="qt1", name="qt1")
                        td = agq_pool.tile([128, 512], dt.bfloat16, tag="qt2", name="qt2")
                        nc.vector.tensor_mul(tg[p0:p0 + 32, :], x1, sn_)
                        nc.vector.tensor_mul(td[p0:p0 + 32, :], x2, cs_)
                        r2t = agq_pool.tile([128, 512], dt.bfloat16, tag="r2t", name="r2t")
                        nc.vector.tensor_add(
                            r2t[p0:p0 + 32, :], tg[p0:p0 + 32, :], td[p0:p0 + 32, :])
                        nc.sync.dma_start(qt_r2[p0 + 32:p0 + 64, tsl], r2t[p0:p0 + 32, :])

                pbp.close()

                # E weights: load during B/C
                for a0 in range(0, 16, 4):
                    nc.sync.dma_start(
                        wo_t[:, a0:a0 + 4, :],
                        wo[a0 * 128:(a0 + 4) * 128, :].rearrange(
                            "(a p) f -> p a f", p=128))

                if upto == 'B':
                    return
                # ========================================================
                # Phase C+E: attention (j-outer), per-chunk AllGather, and
                # o_proj of chunk j-1 overlapped with attention on chunk j
                # ========================================================
                with contextlib.ExitStack() as pc:
                    pt_pool = pc.enter_context(tc.tile_pool(name="pt", bufs=3))
                    sm_pool = pc.enter_context(tc.tile_pool(name="sm", bufs=2))
                    strip_pool = pc.enter_context(tc.tile_pool(name="strips", bufs=2))
                    oo_pool = pc.enter_context(tc.tile_pool(name="oo", bufs=3))
                    ps_s = pc.enter_context(tc.tile_pool(name="ps_s", bufs=2, space="PSUM"))
                    ps_pv = pc.enter_context(tc.tile_pool(name="ps_pv", bufs=2, space="PSUM"))
                    ps_l = pc.enter_context(tc.tile_pool(name="ps_l", bufs=1, space="PSUM"))
                    ps_b = pc.enter_context(tc.tile_pool(name="ps_b", bufs=1, space="PSUM"))
                    ps_e = pc.enter_context(tc.tile_pool(name="ps_e", bufs=2, space="PSUM"))

                    def attn_chunk(j):
                        nk = 4 * j + 4
                        for h in range(HPC):
                            ppv = ps_pv.tile([128, 512], dt.float32, tag="ppv", name="ppv")
                            pl = ps_l.tile([1, 512], dt.float32, tag="pl", name="pl")
                            for ki in range(nk):
                                ksl = slice(ki * 128, (ki + 1) * 128)
                                # diagonal tiles: columns left of the diagonal
                                # are fully masked -> compute only the suffix
                                off = max(0, (ki - 4 * j) * 128)
                                w = 512 - off
                                qs0 = j * 512 + off
                                ps = ps_s.tile([128, 512], dt.float32, tag="ps", name="ps")
                                nc.tensor.matmul(
                                    ps[:, off:], kt_n[h][:, ksl],
                                    qt_n[h][:, qs0:qs0 + w],
                                    start=True, stop=False, skip_group_check=True)
                                nc.tensor.matmul(
                                    ps[:, off:], kpe_t[64 * h:64 * h + 64, ksl],
                                    qt_r2[64 * h:64 * h + 64, qs0:qs0 + w],
                                    start=False, stop=True, skip_group_check=True)
                                if ki >= 4 * j:
                                    nc.vector.tensor_add(
                                        ps[:, off:], ps[:, off:], mask_sb[:, 0, :w])
                                pt = pt_pool.tile([128, 512], dt.bfloat16, tag="pt", name="pt")
                                nc.scalar.activation(pt[:, off:], ps[:, off:], AF.Exp)
                                nc.tensor.matmul(
                                    pl[:, off:], ones_b[:], pt[:, off:],
                                    start=(ki == 0), stop=(ki == nk - 1),
                                    skip_group_check=True)
                                nc.tensor.matmul(
                                    ppv[:, off:], v2_t[ki][:, h * D_V:(h + 1) * D_V],
                                    pt[:, off:],
                                    start=(ki == 0), stop=(ki == nk - 1),
                                    skip_group_check=True)
                            # normalize: attn^T = ppv * (1/l) broadcast
                            rl = sm_pool.tile([1, 512], dt.float32, tag="rl", name="rl")
                            nc.vector.reciprocal_approx_fast(rl[:], pl[:])
                            pb = ps_b.tile([128, 512], dt.float32, tag="pb", name="pb")
                            nc.tensor.matmul(pb[:], ones_f[:], rl[:],
                                             start=True, stop=True,
                                             skip_group_check=True)
                            rb = sm_pool.tile([128, 512], dt.float32, tag="rb", name="rb")
                            nc.vector.tensor_copy(rb[:], pb[:])
                            attn = sm_pool.tile([128, 512], dt.bfloat16, tag="attn", name="attn")
                            nc.vector.tensor_mul(attn[:], ppv[:], rb[:])
                            nc.sync.dma_start(
                                ag2_in[j][h * D_V:(h + 1) * D_V, :], attn[:])
                        nc.gpsimd.collective_compute(
                            "AllGather", mybir.AluOpType.bypass,
                            replica_groups=rg,
                            ins=[ag2_in[j].opt()], outs=[ag2_out[j].opt()])

                    def oproj_chunk(j):
                        jsl = slice(j * 512, (j + 1) * 512)
                        strips = [strip_pool.tile([128, 512], dt.bfloat16,
                                                  tag=f"st{kf}", name=f"st{kf}")
                                  for kf in range(16)]
                        for kf in range(16):
                            nc.sync.dma_start(
                                strips[kf][:],
                                ag2_out[j][kf * 128:(kf + 1) * 128, :])
                        for mt in range(7):
                            msl = slice(mt * 128, (mt + 1) * 128)
                            po = ps_e.tile([128, 512], dt.float32, tag="po", name="po")
                            for kf in range(16):
                                nc.tensor.matmul(
                                    po[:], wo_t[:, kf, msl], strips[kf][:],
                                    start=(kf == 0), stop=(kf == 15),
                                    skip_group_check=True)
                            ot = oo_pool.tile([128, 512], dt.float32, tag="ot", name="ot")
                            nc.scalar.copy(ot[:], po[:])
                            nc.sync.dma_start(out[msl, jsl], ot[:])

                    attn_chunk(0)
                    for j in range(1, 4):
                        attn_chunk(j)
                        oproj_chunk(j - 1)
                    oproj_chunk(3)


def _prep_inputs(hidden_states, positions, W_qkv_a, gamma_q, W_qb, gamma_kv,
                 W_kvb, W_o):
    f32 = np.float32
    perm = np.concatenate([np.arange(0, D_ROPE, 2), np.arange(1, D_ROPE, 2)])
    scale = np.float32(D_QK ** -0.5)

    # A-projection weights: de-interleave k_pe output cols, block layout
    Wa = np.asarray(W_qkv_a, f32).copy()
    Wa[:, QLR + KVLR:] = Wa[:, QLR + KVLR:][:, perm]
    Wa = np.concatenate([Wa, np.zeros((H, 64), f32)], axis=1)  # pad 2112->2176
    # chunk (m, kc) stored so each SBUF partition line is 2KB contiguous:
    # rows (m*56 + kc*8)*128 + p*8 + k8, cols f
    wa_b = (
        Wa.reshape(7, 8, 128, 17, 128)   # [kc, k8, p, m, f]
        .transpose(3, 0, 2, 1, 4)        # [m, kc, p, k8, f]
        .reshape(17 * 56 * 128, 128)
        .astype(BF16)
    )

    # q_b weights: fold gamma_q and score scale, de-interleave rope cols
    Wqb = (np.asarray(W_qb, f32) * np.asarray(gamma_q, f32)[:, None] * scale)
    Wqb = Wqb.reshape(QLR, NH, D_QK)
    Wqb = np.concatenate([Wqb[:, :, :D_NOPE], Wqb[:, :, D_NOPE:][:, :, perm]], axis=2)

    # kv_b weights: fold gamma_kv
    Wkvb = (np.asarray(W_kvb, f32) * np.asarray(gamma_kv, f32)[:, None])
    Wkvb = Wkvb.reshape(KVLR, NH, D_NOPE + D_V)

    Wo = np.asarray(W_o, f32)

    hTf = np.asarray(hidden_states, f32).T.astype(BF16)  # [H, T]

    pos = np.asarray(positions, f32)
    inv_freq = 1.0 / (THETA ** (np.arange(D_ROPE // 2, dtype=f32) / (D_ROPE // 2)))
    freqs = pos[:, None] * inv_freq[None, :]          # [T, 32]
    cos = np.cos(freqs).astype(f32).T                 # [32, T]
    sin = np.sin(freqs).astype(f32).T
    cs = np.concatenate([cos, sin], axis=0)           # [64, T]

    m = np.zeros((4, 128, 512), f32)
    kk = np.arange(128)[:, None]
    qq = np.arange(512)[None, :]
    for oi in range(4):
        m[oi][qq < kk + 128 * oi] = NEG
    masks = m.reshape(4 * 128, 512)

    in_maps = []
    for c in range(NCORES):
        hds = slice(2 * c, 2 * c + 2)
        in_maps.append({
            "hT": np.ascontiguousarray(hTf[:, c * TSH:(c + 1) * TSH]),
            "wa": wa_b,
            "wqb": np.ascontiguousarray(np.concatenate(
                [Wqb[:, 2 * c, :D_NOPE], Wqb[:, 2 * c + 1, :D_NOPE],
                 Wqb[:, 2 * c, D_NOPE:], Wqb[:, 2 * c + 1, D_NOPE:]],
                axis=1)).astype(BF16),
            "wkvb": np.ascontiguousarray(np.concatenate(
                [Wkvb[:, 2 * c, :D_NOPE], Wkvb[:, 2 * c + 1, :D_NOPE],
                 Wkvb[:, 2 * c, D_NOPE:], Wkvb[:, 2 * c + 1, D_NOPE:]],
                axis=1)).astype(BF16),
            "wo": np.ascontiguousarray(
                Wo[:, c * WO_COLS:(c + 1) * WO_COLS]).astype(BF16),
            "cs_sh": np.ascontiguousarray(cs[:, c * TSH:(c + 1) * TSH]),
            "cs_full": cs,
            "masks": masks,
        })
    return in_maps


def kernel(hidden_states, positions, W_qkv_a, gamma_q, W_qb, gamma_kv, W_kvb,
           W_o, _trace=False):
    from concourse.bass_utils import run_bass_kernel_spmd

    if "nc" not in _CACHE:
        _CACHE["nc"] = _build()
    nc = _CACHE["nc"]

    in_maps = _prep_inputs(hidden_states, positions, W_qkv_a, gamma_q, W_qb,
                           gamma_kv, W_kvb, W_o)
    res = run_bass_kernel_spmd(nc, in_maps, list(range(NCORES)), trace=_trace)
    _CACHE["last_result"] = res
    out = np.concatenate(
        [res.results[c]["out"].T for c in range(NCORES)], axis=1)
    return out.astype(np.float32)



# revision 23
# speedup vs baseline: 1.0333x; 1.0333x over previous
"""DeepseekV3 MLA prefill attention on 8 trn2 NeuronCores.

Strategy (single SPMD program, per-core differences live in the input data):
  Phase A: token-split A-projection, computed feature-major
           (qkv^T = W_a^T @ h^T), fused RMSNorm (partition-dim reduce via
           ones-matmul), RoPE on k_pe. gamma and the 1/sqrt(d) score scale
           are folded into the weights on the host; RoPE de-interleave is
           folded into weight column order on the host.
  AG1:     AllGather of normed latents (bf16, feature-major).
  Phase B: per-core head projections Q^T, K^T (feature-major) and V
           (token-major), heads 2c and 2c+1 on core c.
  Phase C: causal attention, S^T = K^T-tiles x Q^T-chunks, exp without
           max-subtraction (scores are O(+-8) by construction), softmax
           denominator via ones-matmul, PV accumulated feature-major,
           block-causal skipping of fully-masked tiles.
  AG2:     AllGather of attention outputs (bf16, feature-major).
  Phase E: column-split o_proj (core c computes output cols 896c..896(c+1)),
           host concatenates.
"""

import numpy as np
import ml_dtypes

T = 2048
H = 7168
NH = 16
D_NOPE = 128
D_ROPE = 64
D_V = 128
D_QK = 192
QLR = 1536
KVLR = 512
THETA = 10000.0
EPS = 1e-6
NCORES = 8
TSH = T // NCORES          # 256 tokens per core
HPC = NH // NCORES         # 2 heads per core
WO_COLS = H // NCORES      # 896 output cols per core
AGF = QLR + KVLR + D_ROPE  # 2112 gathered feature rows
NEG = -30000.0             # mask add, enough to zero bf16/f32 exp

BF16 = ml_dtypes.bfloat16

_CACHE = {}


class _Done(Exception):
    pass


def _build(upto='E'):
    import concourse.bass as bass
    import concourse.mybir as mybir
    import concourse.bacc as bacc
    import concourse.tile as tile

    dt = mybir.dt
    AF = mybir.ActivationFunctionType

    nc = bacc.Bacc(None, target_bir_lowering=False)

    # ---- per-core external inputs -------------------------------------
    hT = nc.declare_dram_parameter("hT", [H, TSH], dt.bfloat16, isOutput=False)
    wa = nc.declare_dram_parameter("wa", [17 * 56 * 128, 128], dt.bfloat16, isOutput=False)
    wqb = nc.declare_dram_parameter("wqb", [QLR, HPC * D_QK], dt.bfloat16, isOutput=False)
    wkvb = nc.declare_dram_parameter("wkvb", [KVLR, HPC * 256], dt.bfloat16, isOutput=False)
    wo = nc.declare_dram_parameter("wo", [NH * D_V, WO_COLS], dt.bfloat16, isOutput=False)
    cs_sh = nc.declare_dram_parameter("cs_sh", [64, TSH], dt.float32, isOutput=False)
    cs_full = nc.declare_dram_parameter("cs_full", [64, T], dt.float32, isOutput=False)
    masks = nc.declare_dram_parameter("masks", [4 * 128, 512], dt.float32, isOutput=False)
    out = nc.declare_dram_parameter("out", [WO_COLS, T], dt.float32, isOutput=True)

    rg = [list(range(NCORES))]

    _build_body(nc, mybir, upto, hT, wa, wqb, wkvb, wo, cs_sh, cs_full,
                masks, out)
    nc.compile()
    return nc


def _build_body(nc, mybir, upto, hT, wa, wqb, wkvb, wo, cs_sh, cs_full,
                masks, out):
    import concourse.tile as tile
    dt = mybir.dt
    AF = mybir.ActivationFunctionType
    rg = [list(range(NCORES))]

    with tile.TileContext(nc) as tc:
        import contextlib

        top = contextlib.ExitStack()
        with top:
            const = top.enter_context(tc.tile_pool(name="const", bufs=1))
            wpool = top.enter_context(tc.tile_pool(name="wpool", bufs=1))
            dram = top.enter_context(tc.tile_pool(name="dram", bufs=1, space="DRAM"))

            ones_b = const.tile([128, 1], dt.bfloat16, tag="ones_b", name="ones_b")
            nc.vector.memset(ones_b[:], 1.0)
            ones_f = const.tile([1, 128], dt.float32, tag="ones_f", name="ones_f")
            nc.vector.memset(ones_f[:], 1.0)
            # tiles allocated up front; DMAs for B/C-phase constants are
            # emitted at point of need so phase A's h/wa stream goes first
            mask_sb = const.tile([128, 4, 512], dt.float32, tag="mask", name="mask")
            # cos/sin replicated to every 32-partition block so DVE ops
            # stay partition-aligned for both heads (loads emitted at m==11)
            csc_r = const.tile([128, T], dt.float32, tag="csc_r", name="csc_r")
            csn_r = const.tile([128, T], dt.float32, tag="csn_r", name="csn_r")
            csc_s = const.tile([32, TSH], dt.float32, tag="csc_s", name="csc_s")
            nc.sync.dma_start(csc_s[:], cs_sh[0:32, :])
            csn_s = const.tile([32, TSH], dt.float32, tag="csn_s", name="csn_s")
            nc.sync.dma_start(csn_s[:], cs_sh[32:64, :])

            # resident weights for phases B and E (loads emitted later)
            wqb_t = wpool.tile([128, 12, HPC * D_QK], dt.bfloat16, tag="wqb", name="wqb")
            wkvb_t = wpool.tile([128, 4, HPC * 256], dt.bfloat16, tag="wkvb", name="wkvb")
            wo_t = wpool.tile([128, 16, WO_COLS], dt.bfloat16, tag="wo", name="wo")

            # collective buffers (AG1 split: kv+k_pe gathered early, q late;
            # AG2 split per head so it overlaps the other head's attention)
            ag1a_in = dram.tile([576, TSH], dt.bfloat16, tag="ag1ai", name="ag1ai")
            ag1a_out = dram.tile([NCORES * 576, TSH], dt.bfloat16, tag="ag1ao", name="ag1ao", addr_space="Shared")
            ag1b_in = dram.tile([QLR, TSH], dt.bfloat16, tag="ag1bi", name="ag1bi")
            ag1b_out = dram.tile([NCORES * QLR, TSH], dt.bfloat16, tag="ag1bo", name="ag1bo", addr_space="Shared")
            # AG2 split per 512-token chunk: chunk j's gather overlaps
            # attention on chunk j+1 and o_proj on chunk j-1
            ag2_in = [dram.tile([2 * D_V, 512], dt.bfloat16, tag=f"ag2i{j}",
                                name=f"ag2i{j}") for j in range(4)]
            ag2_out = [dram.tile([NCORES * 2 * D_V, 512], dt.bfloat16,
                                 tag=f"ag2o{j}", name=f"ag2o{j}",
                                 addr_space="Shared") for j in range(4)]

            # ============================================================
            # Phase A: qkv^T = Wa^T @ h^T   [2112, 256] feature-major
            # ============================================================
            with contextlib.ExitStack() as pa:
                h_pool = pa.enter_context(tc.tile_pool(name="h", bufs=1))
                wa_pool = pa.enter_context(tc.tile_pool(name="wa", bufs=6))
                qkv_pool = pa.enter_context(tc.tile_pool(name="qkv", bufs=1))
                x2_pool = pa.enter_context(tc.tile_pool(name="x2", bufs=3))
                agt_pool = pa.enter_context(tc.tile_pool(name="agt", bufs=3))
                ps_a = pa.enter_context(tc.tile_pool(name="ps_a", bufs=3, space="PSUM"))
                ps_ss = pa.enter_context(tc.tile_pool(name="ps_ss", bufs=1, space="PSUM"))
                ps_bc = pa.enter_context(tc.tile_pool(name="ps_bc", bufs=1, space="PSUM"))

                h_all = h_pool.tile([128, 56, TSH], dt.bfloat16, tag="h_all", name="h_all")
                for a0 in range(0, 56, 7):
                    nc.sync.dma_start(
                        h_all[:, a0:a0 + 7, :],
                        hT[a0 * 128:(a0 + 7) * 128, :].rearrange(
                            "(a p) t -> p a t", p=128))

                qkv = [
                    qkv_pool.tile([128, TSH], dt.float32, tag=f"qkv{m}", name=f"qkv{m}")
                    for m in range(16)
                ]
                kp_raw = qkv_pool.tile([64, TSH], dt.float32, tag="kp_raw", name="kp_raw")
                kp2 = qkv_pool.tile([32, TSH], dt.float32, tag="kp2", name="kp2")

                ss_q = ps_ss.tile([1, TSH], dt.float32, tag="ssq", name="ssq")
                ss_kv = ps_ss.tile([1, TSH], dt.float32, tag="sskv", name="sskv")

                def rstd_bcast(ss, d, name):
                    ms = x2_pool.tile([1, TSH], dt.float32, tag="ms", name="ms")
                    nc.scalar.activation(ms[:], ss[:], AF.Copy, bias=EPS, scale=1.0 / d)
                    inv = x2_pool.tile([1, TSH], dt.float32, tag="inv", name="inv")
                    nc.vector.reciprocal_approx_fast(inv[:], ms[:])
                    rstd = x2_pool.tile([1, TSH], dt.float32, tag="rstd", name="rstd")
                    nc.scalar.activation(rstd[:], inv[:], AF.Sqrt)
                    pb = ps_bc.tile([128, TSH], dt.float32, tag=f"bc{name}", name=f"bc{name}")
                    nc.tensor.matmul(pb[:], ones_f[:], rstd[:], start=True, stop=True)
                    return pb

                pending = [None]

                def emit_ss(m):
                    # squared tile for the RMS partition-sum; deferred into
                    # the next group's matmul stream so the PE never waits
                    # on the Act-eviction -> DVE-square chain
                    x2 = x2_pool.tile([128, TSH], dt.bfloat16, tag="x2", name="x2")
                    nc.vector.tensor_mul(x2[:], qkv[m][:], qkv[m][:])
                    ss = ss_q if m < 12 else ss_kv
                    first = (m == 0) or (m == 12)
                    last = (m == 11) or (m == 15)
                    nc.tensor.matmul(
                        ss[:], ones_b[:], x2[:], start=first, stop=last,
                        skip_group_check=True,
                    )

                for m in list(range(12)) + [12, 13, 14, 15, 16]:
                    mp = 64 if m == 16 else 128
                    psum = ps_a.tile([128, TSH], dt.float32, tag="pa", name="pa")
                    for kc in range(7):
                        chunk = wa_pool.tile([128, 8, 128], dt.bfloat16, tag="wa_c", name="wa_c")
                        r0 = (m * 56 + kc * 8) * 128
                        nc.sync.dma_start(
                            chunk[:],
                            wa[r0:r0 + 1024, :].rearrange("(p a) f -> p a f", a=8),
                        )
                        for k8 in range(8):
                            k = kc * 8 + k8
                            nc.tensor.matmul(
                                psum[:mp, :],
                                chunk[:, k8, :mp],
                                h_all[:, k, :],
                                start=(k == 0),
                                stop=(k == 55),
                                skip_group_check=True,
                            )
                        if kc == 0 and pending[0] is not None:
                            pending[0]()
                            pending[0] = None
                    # evict to f32 SBUF
                    if m < 16:
                        nc.scalar.copy(qkv[m][:], psum[:])
                        if m == 11:
                            emit_ss(m)  # norm needs the full ss_q now
                        else:
                            pending[0] = (lambda m=m: emit_ss(m))
                    else:
                        nc.scalar.copy(kp_raw[:], psum[:64, :])
                        # move the x2 half to base partition 0 for the DVE ops
                        nc.sync.dma_start(kp2[:], kp_raw[32:64, :])
                        if pending[0] is not None:
                            pending[0]()
                            pending[0] = None
                    if m == 11:
                        # q group done: norm q, launch AG1b early so the
                        # gather overlaps the kv-group matmuls
                        bc_q = rstd_bcast(ss_q, QLR, "q")
                        for mm in range(12):
                            agt = agt_pool.tile([128, TSH], dt.bfloat16, tag="agt", name="agt")
                            nc.vector.tensor_mul(agt[:], qkv[mm][:], bc_q[:])
                            nc.sync.dma_start(
                                ag1b_in[mm * 128:(mm + 1) * 128, :], agt[:])
                        nc.gpsimd.collective_compute(
                            "AllGather", mybir.AluOpType.bypass,
                            replica_groups=rg,
                            ins=[ag1b_in.opt()], outs=[ag1b_out.opt()])
                        # B-phase weights: load now, overlapping the gather
                        for a0 in range(0, 12, 4):
                            nc.sync.dma_start(
                                wqb_t[:, a0:a0 + 4, :],
                                wqb[a0 * 128:(a0 + 4) * 128, :].rearrange(
                                    "(a p) f -> p a f", p=128))
                        nc.sync.dma_start(
                            wkvb_t[:], wkvb.rearrange("(a p) f -> p a f", p=128))
                        for i in range(4):
                            nc.sync.dma_start(
                                mask_sb[:, i, :], masks[i * 128:(i + 1) * 128, :])
                        for b4 in range(4):
                            nc.sync.dma_start(
                                csc_r[b4 * 32:(b4 + 1) * 32, :], cs_full[0:32, :])
                            nc.sync.dma_start(
                                csn_r[b4 * 32:(b4 + 1) * 32, :], cs_full[32:64, :])
                    if m == 16:
                        # kv group + k_pe done: norm kv, rope k_pe, launch AG1a
                        bc_kv = rstd_bcast(ss_kv, KVLR, "kv")
                        for mm in range(12, 16):
                            agt = agt_pool.tile([128, TSH], dt.bfloat16, tag="agt", name="agt")
                            nc.vector.tensor_mul(agt[:], qkv[mm][:], bc_kv[:])
                            nc.sync.dma_start(
                                ag1a_in[(mm - 12) * 128:(mm - 11) * 128, :], agt[:])
                        kr1 = agt_pool.tile([32, TSH], dt.bfloat16, tag="kr1", name="kr1")
                        kr2 = agt_pool.tile([32, TSH], dt.bfloat16, tag="kr2", name="kr2")
                        t1 = x2_pool.tile([32, TSH], dt.bfloat16, tag="t1", name="t1")
                        t2 = x2_pool.tile([32, TSH], dt.bfloat16, tag="t2", name="t2")
                        nc.vector.tensor_mul(t1[:], kp_raw[0:32, :], csc_s[:])
                        nc.vector.tensor_mul(t2[:], kp2[:], csn_s[:])
                        nc.vector.tensor_sub(kr1[:], t1[:], t2[:])
                        t3 = x2_pool.tile([32, TSH], dt.bfloat16, tag="t1", name="t1")
                        t4 = x2_pool.tile([32, TSH], dt.bfloat16, tag="t2", name="t2")
                        nc.vector.tensor_mul(t3[:], kp_raw[0:32, :], csn_s[:])
                        nc.vector.tensor_mul(t4[:], kp2[:], csc_s[:])
                        nc.vector.tensor_add(kr2[:], t3[:], t4[:])
                        nc.sync.dma_start(ag1a_in[512:544, :], kr1[:])
                        nc.sync.dma_start(ag1a_in[544:576, :], kr2[:])
                        nc.gpsimd.collective_compute(
                            "AllGather", mybir.AluOpType.bypass,
                            replica_groups=rg,
                            ins=[ag1a_in.opt()], outs=[ag1a_out.opt()])

            if upto == 'A':
                return

            # ============================================================
            # Phase B: Q^T, K^T (feature-major) and V (token-major)
            # ============================================================
            bpools = contextlib.ExitStack()
            with bpools:
                act = bpools.enter_context(tc.tile_pool(name="act", bufs=1))
                agq_pool = bpools.enter_context(tc.tile_pool(name="agq", bufs=6))
                agkv_pool = bpools.enter_context(tc.tile_pool(name="agkv", bufs=2))
                pbp = contextlib.ExitStack()
                ps_q = pbp.enter_context(tc.tile_pool(name="ps_q", bufs=6, space="PSUM"))

                qt_n = [act.tile([128, T], dt.bfloat16, tag=f"qtn{h}", name=f"qtn{h}") for h in range(HPC)]
                # merged rope layout: partitions [64h, 64h+32) = head h x1,
                # [64h+32, 64h+64) = head h x2
                qrw2 = act.tile([128, T], dt.bfloat16, tag="qrw2", name="qrw2")
                qt_r2 = act.tile([128, T], dt.bfloat16, tag="qtr2", name="qtr2")
                kt_n = [act.tile([128, T], dt.bfloat16, tag=f"ktn{h}", name=f"ktn{h}") for h in range(HPC)]
                # k_pe replicated on both 64-partition halves so each
                # head's rope matmul is partition-aligned with qt_r2
                kpe_t = act.tile([128, T], dt.bfloat16, tag="kpet", name="kpet")
                # v2_t[ti]: both heads' V for token tile ti, cols h*128..
                v2_t = [act.tile([128, 2 * D_V], dt.bfloat16, tag=f"v{i}", name=f"v{i}")
                        for i in range(16)]

                for rp in range(4):
                    tsl = slice(rp * 512, (rp + 1) * 512)
                    # --- Q path (two ranks per 512-wide chunk) ---
                    pn0 = ps_q.tile([128, 512], dt.float32, tag="pq", name="pq")
                    pn1 = ps_q.tile([128, 512], dt.float32, tag="pq", name="pq")
                    pr2 = ps_q.tile([128, 512], dt.float32, tag="pq", name="pq")
                    for kq in range(12):
                        aq = agq_pool.tile([128, 512], dt.bfloat16, tag="aq", name="aq")
                        for s in range(2):
                            r = 2 * rp + s
                            nc.sync.dma_start(
                                aq[:, s * TSH:(s + 1) * TSH],
                                ag1b_out[r * QLR + kq * 128: r * QLR + (kq + 1) * 128, :])
                        nc.tensor.matmul(
                            pn0[:], wqb_t[:, kq, 0:128], aq[:],
                            start=(kq == 0), stop=(kq == 11),
                            skip_group_check=True)
                        nc.tensor.matmul(
                            pn1[:], wqb_t[:, kq, 128:256], aq[:],
                            start=(kq == 0), stop=(kq == 11),
                            skip_group_check=True)
                        nc.tensor.matmul(
                            pr2[:], wqb_t[:, kq, 256:384], aq[:],
                            start=(kq == 0), stop=(kq == 11),
                            skip_group_check=True)
                    nc.scalar.copy(qt_n[0][:, tsl], pn0[:])
                    nc.scalar.copy(qt_n[1][:, tsl], pn1[:])
                    nc.scalar.copy(qrw2[:, tsl], pr2[:])
                    # --- RoPE on q for this 512-token chunk; per head all
                    # DVE ops live on partitions [64h, 64h+32) ---
                    for h in range(HPC):
                        p0 = 64 * h
                        x2c = agq_pool.tile([128, 512], dt.bfloat16, tag="x2c", name="x2c")
                        nc.sync.dma_start(
                            x2c[p0:p0 + 32, :], qrw2[p0 + 32:p0 + 64, tsl])
                        x1 = qrw2[p0:p0 + 32, tsl]
                        x2 = x2c[p0:p0 + 32, :]
                        cs_ = csc_r[p0:p0 + 32, tsl]
                        sn_ = csn_r[p0:p0 + 32, tsl]
                        ta = agq_pool.tile([128, 512], dt.bfloat16, tag="qt1", name="qt1")
                        tb = agq_pool.tile([128, 512], dt.bfloat16, tag="qt2", name="qt2")
                        nc.vector.tensor_mul(ta[p0:p0 + 32, :], x1, cs_)
                        nc.vector.tensor_mul(tb[p0:p0 + 32, :], x2, sn_)
                        nc.vector.tensor_sub(
                            qt_r2[p0:p0 + 32, tsl], ta[p0:p0 + 32, :], tb[p0:p0 + 32, :])
                        tg = agq_pool.tile([128, 512], dt.bfloat16, tag="qt1", name="qt1")
                        td = agq_pool.tile([128, 512], dt.bfloat16, tag="qt2", name="qt2")
                        nc.vector.tensor_mul(tg[p0:p0 + 32, :], x1, sn_)
                        nc.vector.tensor_mul(td[p0:p0 + 32, :], x2, cs_)
                        r2t = agq_pool.tile([128, 512], dt.bfloat16, tag="r2t", name="r2t")
                        nc.vector.tensor_add(
                            r2t[p0:p0 + 32, :], tg[p0:p0 + 32, :], td[p0:p0 + 32, :])
                        nc.sync.dma_start(qt_r2[p0 + 32:p0 + 64, tsl], r2t[p0:p0 + 32, :])

                pbp.close()
                pbp = contextlib.ExitStack()
                ps_kv = pbp.enter_context(tc.tile_pool(name="ps_kv", bufs=2, space="PSUM"))

                # KV/V second: AG1a lands after AG1b on the CC queue,
                # so the Q work above covers its transfer
                for rp in range(4):
                    tsl = slice(rp * 512, (rp + 1) * 512)
                    # --- KV path ---
                    akv = [agkv_pool.tile([128, 512], dt.bfloat16, tag=f"akv{kk}", name=f"akv{kk}")
                           for kk in range(4)]
                    for kk in range(4):
                        for s in range(2):
                            r = 2 * rp + s
                            nc.sync.dma_start(
                                akv[kk][:, s * TSH:(s + 1) * TSH],
                                ag1a_out[r * 576 + kk * 128: r * 576 + (kk + 1) * 128, :])
                    for h in range(HPC):
                        pk = ps_kv.tile([128, 512], dt.float32, tag="pkv", name="pkv")
                        for kk in range(4):
                            nc.tensor.matmul(
                                pk[:], wkvb_t[:, kk, h * 128:(h + 1) * 128], akv[kk][:],
                                start=(kk == 0), stop=(kk == 3),
                                skip_group_check=True)
                        nc.scalar.copy(kt_n[h][:, tsl], pk[:])
                    # --- V for both heads at once (wkvb cols 256..512) ---
                    for s4 in range(4):
                        ti = 4 * rp + s4
                        pv = ps_kv.tile([128, 2 * D_V], dt.float32, tag="pv_b", name="pv_b")
                        for kk in range(4):
                            nc.tensor.matmul(
                                pv[:],
                                akv[kk][:, s4 * 128:(s4 + 1) * 128],
                                wkvb_t[:, kk, 256:512],
                                start=(kk == 0), stop=(kk == 3),
                                skip_group_check=True)
                        nc.scalar.copy(v2_t[ti][:], pv[:])
                    # --- shared roped k_pe (both halves) ---
                    for s in range(2):
                        r = 2 * rp + s
                        for hh in range(2):
                            nc.sync.dma_start(
                                kpe_t[64 * hh:64 * hh + 64, r * TSH:(r + 1) * TSH],
                                ag1a_out[r * 576 + 512: r * 576 + 576, :])

                pbp.close()

                # E weights: load during B/C
                for a0 in range(0, 16, 4):
                    nc.sync.dma_start(
                        wo_t[:, a0:a0 + 4, :],
                        wo[a0 * 128:(a0 + 4) * 128, :].rearrange(
                            "(a p) f -> p a f", p=128))

                if upto == 'B':
                    return
                # ========================================================
                # Phase C+E: attention (j-outer), per-chunk AllGather, and
                # o_proj of chunk j-1 overlapped with attention on chunk j
                # ========================================================
                with contextlib.ExitStack() as pc:
                    pt_pool = pc.enter_context(tc.tile_pool(name="pt", bufs=3))
                    sm_pool = pc.enter_context(tc.tile_pool(name="sm", bufs=2))
                    strip_pool = pc.enter_context(tc.tile_pool(name="strips", bufs=2))
                    oo_pool = pc.enter_context(tc.tile_pool(name="oo", bufs=3))
                    ps_s = pc.enter_context(tc.tile_pool(name="ps_s", bufs=2, space="PSUM"))
                    ps_pv = pc.enter_context(tc.tile_pool(name="ps_pv", bufs=2, space="PSUM"))
                    ps_l = pc.enter_context(tc.tile_pool(name="ps_l", bufs=1, space="PSUM"))
                    ps_b = pc.enter_context(tc.tile_pool(name="ps_b", bufs=1, space="PSUM"))
                    ps_e = pc.enter_context(tc.tile_pool(name="ps_e", bufs=2, space="PSUM"))

                    def attn_chunk(j):
                        nk = 4 * j + 4
                        for h in range(HPC):
                            ppv = ps_pv.tile([128, 512], dt.float32, tag="ppv", name="ppv")
                            pl = ps_l.tile([1, 512], dt.float32, tag="pl", name="pl")
                            for ki in range(nk):
                                ksl = slice(ki * 128, (ki + 1) * 128)
                                # diagonal tiles: columns left of the diagonal
                                # are fully masked -> compute only the suffix
                                off = max(0, (ki - 4 * j) * 128)
                                w = 512 - off
                                qs0 = j * 512 + off
                                ps = ps_s.tile([128, 512], dt.float32, tag="ps", name="ps")
                                nc.tensor.matmul(
                                    ps[:, off:], kt_n[h][:, ksl],
                                    qt_n[h][:, qs0:qs0 + w],
                                    start=True, stop=False, skip_group_check=True)
                                nc.tensor.matmul(
                                    ps[:, off:], kpe_t[64 * h:64 * h + 64, ksl],
                                    qt_r2[64 * h:64 * h + 64, qs0:qs0 + w],
                                    start=False, stop=True, skip_group_check=True)
                                if ki >= 4 * j:
                                    nc.vector.tensor_add(
                                        ps[:, off:], ps[:, off:], mask_sb[:, 0, :w])
                                pt = pt_pool.tile([128, 512], dt.bfloat16, tag="pt", name="pt")
                                nc.scalar.activation(pt[:, off:], ps[:, off:], AF.Exp)
                                nc.tensor.matmul(
                                    pl[:, off:], ones_b[:], pt[:, off:],
                                    start=(ki == 0), stop=(ki == nk - 1),
                                    skip_group_check=True)
                                nc.tensor.matmul(
                                    ppv[:, off:], v2_t[ki][:, h * D_V:(h + 1) * D_V],
                                    pt[:, off:],
                                    start=(ki == 0), stop=(ki == nk - 1),
                                    skip_group_check=True)
                            # normalize: attn^T = ppv * (1/l) broadcast
                            rl = sm_pool.tile([1, 512], dt.float32, tag="rl", name="rl")
                            nc.vector.reciprocal_approx_fast(rl[:], pl[:])
                            pb = ps_b.tile([128, 512], dt.float32, tag="pb", name="pb")
                            nc.tensor.matmul(pb[:], ones_f[:], rl[:],
                                             start=True, stop=True,
                                             skip_group_check=True)
                            rb = sm_pool.tile([128, 512], dt.float32, tag="rb", name="rb")
                            nc.vector.tensor_copy(rb[:], pb[:])
                            attn = sm_pool.tile([128, 512], dt.bfloat16, tag="attn", name="attn")
                            nc.vector.tensor_mul(attn[:], ppv[:], rb[:])
                            nc.sync.dma_start(
                                ag2_in[j][h * D_V:(h + 1) * D_V, :], attn[:])
                        nc.gpsimd.collective_compute(
                            "AllGather", mybir.AluOpType.bypass,
                            replica_groups=rg,
                            ins=[ag2_in[j].opt()], outs=[ag2_out[j].opt()])

                    def oproj_chunk(j):
                        jsl = slice(j * 512, (j + 1) * 512)
                        strips = [strip_pool.tile([128, 512], dt.bfloat16,
                                                  tag=f"st{kf}", name=f"st{kf}")
                                  for kf in range(16)]
                        for kf in range(16):
                            nc.sync.dma_start(
                                strips[kf][:],
                                ag2_out[j][kf * 128:(kf + 1) * 128, :])
                        for mt in range(7):
                            msl = slice(mt * 128, (mt + 1) * 128)
                            po = ps_e.tile([128, 512], dt.float32, tag="po", name="po")
                            for kf in range(16):
                                nc.tensor.matmul(
                                    po[:], wo_t[:, kf, msl], strips[kf][:],
                                    start=(kf == 0), stop=(kf == 15),
                                    skip_group_check=True)
                            ot = oo_pool.tile([128, 512], dt.float32, tag="ot", name="ot")
                            nc.scalar.copy(ot[:], po[:])
                            nc.sync.dma_start(out[msl, jsl], ot[:])

                    attn_chunk(0)
                    for j in range(1, 4):
                        attn_chunk(j)
                        oproj_chunk(j - 1)
                    oproj_chunk(3)


def _prep_inputs(hidden_states, positions, W_qkv_a, gamma_q, W_qb, gamma_kv,
                 W_kvb, W_o):
    f32 = np.float32
    perm = np.concatenate([np.arange(0, D_ROPE, 2), np.arange(1, D_ROPE, 2)])
    scale = np.float32(D_QK ** -0.5)

    # A-projection weights: de-interleave k_pe output cols, block layout
    Wa = np.asarray(W_qkv_a, f32).copy()
    Wa[:, QLR + KVLR:] = Wa[:, QLR + KVLR:][:, perm]
    Wa = np.concatenate([Wa, np.zeros((H, 64), f32)], axis=1)  # pad 2112->2176
    # chunk (m, kc) stored so each SBUF partition line is 2KB contiguous:
    # rows (m*56 + kc*8)*128 + p*8 + k8, cols f
    wa_b = (
        Wa.reshape(7, 8, 128, 17, 128)   # [kc, k8, p, m, f]
        .transpose(3, 0, 2, 1, 4)        # [m, kc, p, k8, f]
        .reshape(17 * 56 * 128, 128)
        .astype(BF16)
    )

    # q_b weights: fold gamma_q and score scale, de-interleave rope cols
    Wqb = (np.asarray(W_qb, f32) * np.asarray(gamma_q, f32)[:, None] * scale)
    Wqb = Wqb.reshape(QLR, NH, D_QK)
    Wqb = np.concatenate([Wqb[:, :, :D_NOPE], Wqb[:, :, D_NOPE:][:, :, perm]], axis=2)

    # kv_b weights: fold gamma_kv
    Wkvb = (np.asarray(W_kvb, f32) * np.asarray(gamma_kv, f32)[:, None])
    Wkvb = Wkvb.reshape(KVLR, NH, D_NOPE + D_V)

    Wo = np.asarray(W_o, f32)

    hTf = np.asarray(hidden_states, f32).T.astype(BF16)  # [H, T]

    pos = np.asarray(positions, f32)
    inv_freq = 1.0 / (THETA ** (np.arange(D_ROPE // 2, dtype=f32) / (D_ROPE // 2)))
    freqs = pos[:, None] * inv_freq[None, :]          # [T, 32]
    cos = np.cos(freqs).astype(f32).T                 # [32, T]
    sin = np.sin(freqs).astype(f32).T
    cs = np.concatenate([cos, sin], axis=0)           # [64, T]

    m = np.zeros((4, 128, 512), f32)
    kk = np.arange(128)[:, None]
    qq = np.arange(512)[None, :]
    for oi in range(4):
        m[oi][qq < kk + 128 * oi] = NEG
    masks = m.reshape(4 * 128, 512)

    in_maps = []
    for c in range(NCORES):
        hds = slice(2 * c, 2 * c + 2)
        in_maps.append({
            "hT": np.ascontiguousarray(hTf[:, c * TSH:(c + 1) * TSH]),
            "wa": wa_b,
            "wqb": np.ascontiguousarray(np.concatenate(
                [Wqb[:, 2 * c, :D_NOPE], Wqb[:, 2 * c + 1, :D_NOPE],
                 Wqb[:, 2 * c, D_NOPE:], Wqb[:, 2 * c + 1, D_NOPE:]],
                axis=1)).astype(BF16),
            "wkvb": np.ascontiguousarray(np.concatenate(
                [Wkvb[:, 2 * c, :D_NOPE], Wkvb[:, 2 * c + 1, :D_NOPE],
                 Wkvb[:, 2 * c, D_NOPE:], Wkvb[:, 2 * c + 1, D_NOPE:]],
                axis=1)).astype(BF16),
            "wo": np.ascontiguousarray(
                Wo[:, c * WO_COLS:(c + 1) * WO_COLS]).astype(BF16),
            "cs_sh": np.ascontiguousarray(cs[:, c * TSH:(c + 1) * TSH]),
            "cs_full": cs,
            "masks": masks,
        })
    return in_maps


def kernel(hidden_states, positions, W_qkv_a, gamma_q, W_qb, gamma_kv, W_kvb,
           W_o, _trace=False):
    from concourse.bass_utils import run_bass_kernel_spmd

    if "nc" not in _CACHE:
        _CACHE["nc"] = _build()
    nc = _CACHE["nc"]

    in_maps = _prep_inputs(hidden_states, positions, W_qkv_a, gamma_q, W_qb,
                           gamma_kv, W_kvb, W_o)
    res = run_bass_kernel_spmd(nc, in_maps, list(range(NCORES)), trace=_trace)
    _CACHE["last_result"] = res
    out = np.concatenate(
        [res.results[c]["out"].T for c in range(NCORES)], axis=1)
    return out.astype(np.float32)



# revision 24
# speedup vs baseline: 1.0758x; 1.0412x over previous
"""DeepseekV3 MLA prefill attention on 8 trn2 NeuronCores.

Strategy (single SPMD program, per-core differences live in the input data):
  Phase A: token-split A-projection, computed feature-major
           (qkv^T = W_a^T @ h^T), fused RMSNorm (partition-dim reduce via
           ones-matmul), RoPE on k_pe. gamma and the 1/sqrt(d) score scale
           are folded into the weights on the host; RoPE de-interleave is
           folded into weight column order on the host.
  AG1:     AllGather of normed latents (bf16, feature-major).
  Phase B: per-core head projections Q^T, K^T (feature-major) and V
           (token-major), heads 2c and 2c+1 on core c.
  Phase C: causal attention, S^T = K^T-tiles x Q^T-chunks, exp without
           max-subtraction (scores are O(+-8) by construction), softmax
           denominator via ones-matmul, PV accumulated feature-major,
           block-causal skipping of fully-masked tiles.
  AG2:     AllGather of attention outputs (bf16, feature-major).
  Phase E: column-split o_proj (core c computes output cols 896c..896(c+1)),
           host concatenates.
"""

import numpy as np
import ml_dtypes

T = 2048
H = 7168
NH = 16
D_NOPE = 128
D_ROPE = 64
D_V = 128
D_QK = 192
QLR = 1536
KVLR = 512
THETA = 10000.0
EPS = 1e-6
NCORES = 8
TSH = T // NCORES          # 256 tokens per core
HPC = NH // NCORES         # 2 heads per core
WO_COLS = H // NCORES      # 896 output cols per core
AGF = QLR + KVLR + D_ROPE  # 2112 gathered feature rows
NEG = -30000.0             # mask add, enough to zero bf16/f32 exp

BF16 = ml_dtypes.bfloat16

_CACHE = {}


class _Done(Exception):
    pass


def _build(upto='E'):
    import concourse.bass as bass
    import concourse.mybir as mybir
    import concourse.bacc as bacc
    import concourse.tile as tile

    dt = mybir.dt
    AF = mybir.ActivationFunctionType

    nc = bacc.Bacc(None, target_bir_lowering=False)

    # ---- per-core external inputs -------------------------------------
    hT = nc.declare_dram_parameter("hT", [H, TSH], dt.bfloat16, isOutput=False)
    wa = nc.declare_dram_parameter("wa", [17 * 56 * 128, 128], dt.bfloat16, isOutput=False)
    wqb = nc.declare_dram_parameter("wqb", [QLR, HPC * D_QK], dt.bfloat16, isOutput=False)
    wkvb = nc.declare_dram_parameter("wkvb", [KVLR, HPC * 256], dt.bfloat16, isOutput=False)
    wo = nc.declare_dram_parameter("wo", [NH * D_V, WO_COLS], dt.bfloat16, isOutput=False)
    cs_sh = nc.declare_dram_parameter("cs_sh", [64, TSH], dt.float32, isOutput=False)
    cs_full = nc.declare_dram_parameter("cs_full", [64, T], dt.float32, isOutput=False)
    masks = nc.declare_dram_parameter("masks", [4 * 128, 512], dt.float32, isOutput=False)
    out = nc.declare_dram_parameter("out", [WO_COLS, T], dt.float32, isOutput=True)

    rg = [list(range(NCORES))]

    _build_body(nc, mybir, upto, hT, wa, wqb, wkvb, wo, cs_sh, cs_full,
                masks, out)
    nc.compile()
    return nc


def _build_body(nc, mybir, upto, hT, wa, wqb, wkvb, wo, cs_sh, cs_full,
                masks, out):
    import concourse.tile as tile
    dt = mybir.dt
    AF = mybir.ActivationFunctionType
    rg = [list(range(NCORES))]

    with tile.TileContext(nc) as tc:
        import contextlib

        top = contextlib.ExitStack()
        with top:
            const = top.enter_context(tc.tile_pool(name="const", bufs=1))
            wpool = top.enter_context(tc.tile_pool(name="wpool", bufs=1))
            dram = top.enter_context(tc.tile_pool(name="dram", bufs=1, space="DRAM"))

            ones_b = const.tile([128, 1], dt.bfloat16, tag="ones_b", name="ones_b")
            nc.vector.memset(ones_b[:], 1.0)
            ones_f = const.tile([1, 128], dt.float16, tag="ones_f", name="ones_f")
            nc.vector.memset(ones_f[:], 1.0)
            # tiles allocated up front; DMAs for B/C-phase constants are
            # emitted at point of need so phase A's h/wa stream goes first
            mask_sb = const.tile([128, 4, 512], dt.float32, tag="mask", name="mask")
            # cos/sin replicated to every 32-partition block so DVE ops
            # stay partition-aligned for both heads (loads emitted at m==11)
            csc_r = const.tile([128, T], dt.float32, tag="csc_r", name="csc_r")
            csn_r = const.tile([128, T], dt.float32, tag="csn_r", name="csn_r")
            csc_s = const.tile([32, TSH], dt.float32, tag="csc_s", name="csc_s")
            nc.sync.dma_start(csc_s[:], cs_sh[0:32, :])
            csn_s = const.tile([32, TSH], dt.float32, tag="csn_s", name="csn_s")
            nc.sync.dma_start(csn_s[:], cs_sh[32:64, :])

            # resident weights for phases B and E (loads emitted later)
            wqb_t = wpool.tile([128, 12, HPC * D_QK], dt.bfloat16, tag="wqb", name="wqb")
            wkvb_t = wpool.tile([128, 4, HPC * 256], dt.bfloat16, tag="wkvb", name="wkvb")
            wo_t = wpool.tile([128, 16, WO_COLS], dt.bfloat16, tag="wo", name="wo")

            # collective buffers (AG1 split: kv+k_pe gathered early, q late;
            # AG2 split per head so it overlaps the other head's attention)
            ag1a_in = dram.tile([576, TSH], dt.bfloat16, tag="ag1ai", name="ag1ai")
            ag1a_out = dram.tile([NCORES * 576, TSH], dt.bfloat16, tag="ag1ao", name="ag1ao", addr_space="Shared")
            ag1b_in = dram.tile([QLR, TSH], dt.bfloat16, tag="ag1bi", name="ag1bi")
            ag1b_out = dram.tile([NCORES * QLR, TSH], dt.bfloat16, tag="ag1bo", name="ag1bo", addr_space="Shared")
            # AG2 split per 512-token chunk: chunk j's gather overlaps
            # attention on chunk j+1 and o_proj on chunk j-1
            ag2_in = [dram.tile([2 * D_V, 512], dt.bfloat16, tag=f"ag2i{j}",
                                name=f"ag2i{j}") for j in range(4)]
            ag2_out = [dram.tile([NCORES * 2 * D_V, 512], dt.bfloat16,
                                 tag=f"ag2o{j}", name=f"ag2o{j}",
                                 addr_space="Shared") for j in range(4)]

            # ============================================================
            # Phase A: qkv^T = Wa^T @ h^T   [2112, 256] feature-major
            # ============================================================
            with contextlib.ExitStack() as pa:
                h_pool = pa.enter_context(tc.tile_pool(name="h", bufs=1))
                wa_pool = pa.enter_context(tc.tile_pool(name="wa", bufs=6))
                qkv_pool = pa.enter_context(tc.tile_pool(name="qkv", bufs=1))
                x2_pool = pa.enter_context(tc.tile_pool(name="x2", bufs=3))
                agt_pool = pa.enter_context(tc.tile_pool(name="agt", bufs=3))
                ps_a = pa.enter_context(tc.tile_pool(name="ps_a", bufs=3, space="PSUM"))
                ps_ss = pa.enter_context(tc.tile_pool(name="ps_ss", bufs=1, space="PSUM"))
                ps_bc = pa.enter_context(tc.tile_pool(name="ps_bc", bufs=1, space="PSUM"))

                h_all = h_pool.tile([128, 56, TSH], dt.bfloat16, tag="h_all", name="h_all")
                for a0 in range(0, 56, 7):
                    nc.sync.dma_start(
                        h_all[:, a0:a0 + 7, :],
                        hT[a0 * 128:(a0 + 7) * 128, :].rearrange(
                            "(a p) t -> p a t", p=128))

                qkv = [
                    qkv_pool.tile([128, TSH], dt.float32, tag=f"qkv{m}", name=f"qkv{m}")
                    for m in range(16)
                ]
                kp_raw = qkv_pool.tile([64, TSH], dt.float32, tag="kp_raw", name="kp_raw")
                kp2 = qkv_pool.tile([32, TSH], dt.float32, tag="kp2", name="kp2")

                ss_q = ps_ss.tile([1, TSH], dt.float32, tag="ssq", name="ssq")
                ss_kv = ps_ss.tile([1, TSH], dt.float32, tag="sskv", name="sskv")

                def rstd_bcast(ss, d, name):
                    ms = x2_pool.tile([1, TSH], dt.float32, tag="ms", name="ms")
                    nc.scalar.activation(ms[:], ss[:], AF.Copy, bias=EPS, scale=1.0 / d)
                    inv = x2_pool.tile([1, TSH], dt.float32, tag="inv", name="inv")
                    nc.vector.reciprocal_approx_fast(inv[:], ms[:])
                    rstd = x2_pool.tile([1, TSH], dt.float16, tag="rstd", name="rstd")
                    nc.scalar.activation(rstd[:], inv[:], AF.Sqrt)
                    pb = ps_bc.tile([128, TSH], dt.float32, tag=f"bc{name}", name=f"bc{name}")
                    nc.tensor.matmul(pb[:], ones_f[:], rstd[:], start=True, stop=True)
                    return pb

                pending = [None]

                def emit_ss(m):
                    # squared tile for the RMS partition-sum; deferred into
                    # the next group's matmul stream so the PE never waits
                    # on the Act-eviction -> DVE-square chain
                    x2 = x2_pool.tile([128, TSH], dt.bfloat16, tag="x2", name="x2")
                    nc.vector.tensor_mul(x2[:], qkv[m][:], qkv[m][:])
                    ss = ss_q if m < 12 else ss_kv
                    first = (m == 0) or (m == 12)
                    last = (m == 11) or (m == 15)
                    nc.tensor.matmul(
                        ss[:], ones_b[:], x2[:], start=first, stop=last,
                        skip_group_check=True,
                    )

                for m in list(range(12)) + [12, 13, 14, 15, 16]:
                    mp = 64 if m == 16 else 128
                    psum = ps_a.tile([128, TSH], dt.float32, tag="pa", name="pa")
                    for kc in range(7):
                        chunk = wa_pool.tile([128, 8, 128], dt.bfloat16, tag="wa_c", name="wa_c")
                        r0 = (m * 56 + kc * 8) * 128
                        nc.sync.dma_start(
                            chunk[:],
                            wa[r0:r0 + 1024, :].rearrange("(p a) f -> p a f", a=8),
                        )
                        for k8 in range(8):
                            k = kc * 8 + k8
                            nc.tensor.matmul(
                                psum[:mp, :],
                                chunk[:, k8, :mp],
                                h_all[:, k, :],
                                start=(k == 0),
                                stop=(k == 55),
                                skip_group_check=True,
                            )
                        if kc == 0 and pending[0] is not None:
                            pending[0]()
                            pending[0] = None
                    # evict to f32 SBUF
                    if m < 16:
                        nc.scalar.copy(qkv[m][:], psum[:])
                        if m == 11:
                            emit_ss(m)  # norm needs the full ss_q now
                        else:
                            pending[0] = (lambda m=m: emit_ss(m))
                    else:
                        nc.scalar.copy(kp_raw[:], psum[:64, :])
                        # move the x2 half to base partition 0 for the DVE ops
                        nc.sync.dma_start(kp2[:], kp_raw[32:64, :])
                        if pending[0] is not None:
                            pending[0]()
                            pending[0] = None
                    if m == 11:
                        # q group done: norm q, launch AG1b early so the
                        # gather overlaps the kv-group matmuls
                        bc_q = rstd_bcast(ss_q, QLR, "q")
                        for mm in range(12):
                            agt = agt_pool.tile([128, TSH], dt.bfloat16, tag="agt", name="agt")
                            nc.vector.tensor_mul(agt[:], qkv[mm][:], bc_q[:])
                            nc.sync.dma_start(
                                ag1b_in[mm * 128:(mm + 1) * 128, :], agt[:])
                        nc.gpsimd.collective_compute(
                            "AllGather", mybir.AluOpType.bypass,
                            replica_groups=rg,
                            ins=[ag1b_in.opt()], outs=[ag1b_out.opt()])
                        # B-phase weights: load now, overlapping the gather
                        for a0 in range(0, 12, 4):
                            nc.sync.dma_start(
                                wqb_t[:, a0:a0 + 4, :],
                                wqb[a0 * 128:(a0 + 4) * 128, :].rearrange(
                                    "(a p) f -> p a f", p=128))
                        nc.sync.dma_start(
                            wkvb_t[:], wkvb.rearrange("(a p) f -> p a f", p=128))
                        for i in range(4):
                            nc.sync.dma_start(
                                mask_sb[:, i, :], masks[i * 128:(i + 1) * 128, :])
                        for b4 in range(4):
                            nc.sync.dma_start(
                                csc_r[b4 * 32:(b4 + 1) * 32, :], cs_full[0:32, :])
                            nc.sync.dma_start(
                                csn_r[b4 * 32:(b4 + 1) * 32, :], cs_full[32:64, :])
                    if m == 16:
                        # kv group + k_pe done: norm kv, rope k_pe, launch AG1a
                        bc_kv = rstd_bcast(ss_kv, KVLR, "kv")
                        for mm in range(12, 16):
                            agt = agt_pool.tile([128, TSH], dt.bfloat16, tag="agt", name="agt")
                            nc.vector.tensor_mul(agt[:], qkv[mm][:], bc_kv[:])
                            nc.sync.dma_start(
                                ag1a_in[(mm - 12) * 128:(mm - 11) * 128, :], agt[:])
                        kr1 = agt_pool.tile([32, TSH], dt.bfloat16, tag="kr1", name="kr1")
                        kr2 = agt_pool.tile([32, TSH], dt.bfloat16, tag="kr2", name="kr2")
                        t1 = x2_pool.tile([32, TSH], dt.bfloat16, tag="t1", name="t1")
                        t2 = x2_pool.tile([32, TSH], dt.bfloat16, tag="t2", name="t2")
                        nc.vector.tensor_mul(t1[:], kp_raw[0:32, :], csc_s[:])
                        nc.vector.tensor_mul(t2[:], kp2[:], csn_s[:])
                        nc.vector.tensor_sub(kr1[:], t1[:], t2[:])
                        t3 = x2_pool.tile([32, TSH], dt.bfloat16, tag="t1", name="t1")
                        t4 = x2_pool.tile([32, TSH], dt.bfloat16, tag="t2", name="t2")
                        nc.vector.tensor_mul(t3[:], kp_raw[0:32, :], csn_s[:])
                        nc.vector.tensor_mul(t4[:], kp2[:], csc_s[:])
                        nc.vector.tensor_add(kr2[:], t3[:], t4[:])
                        nc.sync.dma_start(ag1a_in[512:544, :], kr1[:])
                        nc.sync.dma_start(ag1a_in[544:576, :], kr2[:])
                        nc.gpsimd.collective_compute(
                            "AllGather", mybir.AluOpType.bypass,
                            replica_groups=rg,
                            ins=[ag1a_in.opt()], outs=[ag1a_out.opt()])

            if upto == 'A':
                return

            # ============================================================
            # Phase B: Q^T, K^T (feature-major) and V (token-major)
            # ============================================================
            bpools = contextlib.ExitStack()
            with bpools:
                act = bpools.enter_context(tc.tile_pool(name="act", bufs=1))
                agq_pool = bpools.enter_context(tc.tile_pool(name="agq", bufs=6))
                agkv_pool = bpools.enter_context(tc.tile_pool(name="agkv", bufs=2))
                pbp = contextlib.ExitStack()
                ps_q = pbp.enter_context(tc.tile_pool(name="ps_q", bufs=6, space="PSUM"))

                qt_n = [act.tile([128, T], dt.bfloat16, tag=f"qtn{h}", name=f"qtn{h}") for h in range(HPC)]
                # merged rope layout: partitions [64h, 64h+32) = head h x1,
                # [64h+32, 64h+64) = head h x2
                qrw2 = act.tile([128, T], dt.bfloat16, tag="qrw2", name="qrw2")
                qt_r2 = act.tile([128, T], dt.bfloat16, tag="qtr2", name="qtr2")
                kt_n = [act.tile([128, T], dt.bfloat16, tag=f"ktn{h}", name=f"ktn{h}") for h in range(HPC)]
                # k_pe replicated on both 64-partition halves so each
                # head's rope matmul is partition-aligned with qt_r2
                kpe_t = act.tile([128, T], dt.bfloat16, tag="kpet", name="kpet")
                # v2_t[ti]: both heads' V for token tile ti, cols h*128..
                v2_t = [act.tile([128, 2 * D_V], dt.bfloat16, tag=f"v{i}", name=f"v{i}")
                        for i in range(16)]

                for rp in range(4):
                    tsl = slice(rp * 512, (rp + 1) * 512)
                    # --- Q path (two ranks per 512-wide chunk) ---
                    pn0 = ps_q.tile([128, 512], dt.float32, tag="pq", name="pq")
                    pn1 = ps_q.tile([128, 512], dt.float32, tag="pq", name="pq")
                    pr2 = ps_q.tile([128, 512], dt.float32, tag="pq", name="pq")
                    for kq in range(12):
                        aq = agq_pool.tile([128, 512], dt.bfloat16, tag="aq", name="aq")
                        for s in range(2):
                            r = 2 * rp + s
                            nc.sync.dma_start(
                                aq[:, s * TSH:(s + 1) * TSH],
                                ag1b_out[r * QLR + kq * 128: r * QLR + (kq + 1) * 128, :])
                        nc.tensor.matmul(
                            pn0[:], wqb_t[:, kq, 0:128], aq[:],
                            start=(kq == 0), stop=(kq == 11),
                            skip_group_check=True)
                        nc.tensor.matmul(
                            pn1[:], wqb_t[:, kq, 128:256], aq[:],
                            start=(kq == 0), stop=(kq == 11),
                            skip_group_check=True)
                        nc.tensor.matmul(
                            pr2[:], wqb_t[:, kq, 256:384], aq[:],
                            start=(kq == 0), stop=(kq == 11),
                            skip_group_check=True)
                    nc.scalar.copy(qt_n[0][:, tsl], pn0[:])
                    nc.scalar.copy(qt_n[1][:, tsl], pn1[:])
                    nc.scalar.copy(qrw2[:, tsl], pr2[:])
                    # --- RoPE on q for this 512-token chunk; per head all
                    # DVE ops live on partitions [64h, 64h+32) ---
                    for h in range(HPC):
                        p0 = 64 * h
                        x2c = agq_pool.tile([128, 512], dt.bfloat16, tag="x2c", name="x2c")
                        nc.sync.dma_start(
                            x2c[p0:p0 + 32, :], qrw2[p0 + 32:p0 + 64, tsl])
                        x1 = qrw2[p0:p0 + 32, tsl]
                        x2 = x2c[p0:p0 + 32, :]
                        cs_ = csc_r[p0:p0 + 32, tsl]
                        sn_ = csn_r[p0:p0 + 32, tsl]
                        ta = agq_pool.tile([128, 512], dt.bfloat16, tag="qt1", name="qt1")
                        tb = agq_pool.tile([128, 512], dt.bfloat16, tag="qt2", name="qt2")
                        nc.vector.tensor_mul(ta[p0:p0 + 32, :], x1, cs_)
                        nc.vector.tensor_mul(tb[p0:p0 + 32, :], x2, sn_)
                        nc.vector.tensor_sub(
                            qt_r2[p0:p0 + 32, tsl], ta[p0:p0 + 32, :], tb[p0:p0 + 32, :])
                        tg = agq_pool.tile([128, 512], dt.bfloat16, tag="qt1", name="qt1")
                        td = agq_pool.tile([128, 512], dt.bfloat16, tag="qt2", name="qt2")
                        nc.vector.tensor_mul(tg[p0:p0 + 32, :], x1, sn_)
                        nc.vector.tensor_mul(td[p0:p0 + 32, :], x2, cs_)
                        r2t = agq_pool.tile([128, 512], dt.bfloat16, tag="r2t", name="r2t")
                        nc.vector.tensor_add(
                            r2t[p0:p0 + 32, :], tg[p0:p0 + 32, :], td[p0:p0 + 32, :])
                        nc.sync.dma_start(qt_r2[p0 + 32:p0 + 64, tsl], r2t[p0:p0 + 32, :])

                pbp.close()
                pbp = contextlib.ExitStack()
                ps_kv = pbp.enter_context(tc.tile_pool(name="ps_kv", bufs=2, space="PSUM"))

                # KV/V second: AG1a lands after AG1b on the CC queue,
                # so the Q work above covers its transfer
                for rp in range(4):
                    tsl = slice(rp * 512, (rp + 1) * 512)
                    # --- KV path ---
                    akv = [agkv_pool.tile([128, 512], dt.bfloat16, tag=f"akv{kk}", name=f"akv{kk}")
                           for kk in range(4)]
                    for kk in range(4):
                        for s in range(2):
                            r = 2 * rp + s
                            nc.sync.dma_start(
                                akv[kk][:, s * TSH:(s + 1) * TSH],
                                ag1a_out[r * 576 + kk * 128: r * 576 + (kk + 1) * 128, :])
                    for h in range(HPC):
                        pk = ps_kv.tile([128, 512], dt.float32, tag="pkv", name="pkv")
                        for kk in range(4):
                            nc.tensor.matmul(
                                pk[:], wkvb_t[:, kk, h * 128:(h + 1) * 128], akv[kk][:],
                                start=(kk == 0), stop=(kk == 3),
                                skip_group_check=True)
                        nc.scalar.copy(kt_n[h][:, tsl], pk[:])
                    # --- V for both heads at once (wkvb cols 256..512) ---
                    for s4 in range(4):
                        ti = 4 * rp + s4
                        pv = ps_kv.tile([128, 2 * D_V], dt.float32, tag="pv_b", name="pv_b")
                        for kk in range(4):
                            nc.tensor.matmul(
                                pv[:],
                                akv[kk][:, s4 * 128:(s4 + 1) * 128],
                                wkvb_t[:, kk, 256:512],
                                start=(kk == 0), stop=(kk == 3),
                                skip_group_check=True)
                        nc.scalar.copy(v2_t[ti][:], pv[:])
                    # --- shared roped k_pe (both halves) ---
                    for s in range(2):
                        r = 2 * rp + s
                        for hh in range(2):
                            nc.sync.dma_start(
                                kpe_t[64 * hh:64 * hh + 64, r * TSH:(r + 1) * TSH],
                                ag1a_out[r * 576 + 512: r * 576 + 576, :])

                pbp.close()

                # E weights: load during B/C
                for a0 in range(0, 16, 4):
                    nc.sync.dma_start(
                        wo_t[:, a0:a0 + 4, :],
                        wo[a0 * 128:(a0 + 4) * 128, :].rearrange(
                            "(a p) f -> p a f", p=128))

                if upto == 'B':
                    return
                # ========================================================
                # Phase C+E: attention (j-outer), per-chunk AllGather, and
                # o_proj of chunk j-1 overlapped with attention on chunk j
                # ========================================================
                with contextlib.ExitStack() as pc:
                    pt_pool = pc.enter_context(tc.tile_pool(name="pt", bufs=3))
                    sm_pool = pc.enter_context(tc.tile_pool(name="sm", bufs=2))
                    strip_pool = pc.enter_context(tc.tile_pool(name="strips", bufs=2))
                    oo_pool = pc.enter_context(tc.tile_pool(name="oo", bufs=3))
                    ps_s = pc.enter_context(tc.tile_pool(name="ps_s", bufs=2, space="PSUM"))
                    ps_pv = pc.enter_context(tc.tile_pool(name="ps_pv", bufs=2, space="PSUM"))
                    ps_l = pc.enter_context(tc.tile_pool(name="ps_l", bufs=1, space="PSUM"))
                    ps_b = pc.enter_context(tc.tile_pool(name="ps_b", bufs=1, space="PSUM"))
                    ps_e = pc.enter_context(tc.tile_pool(name="ps_e", bufs=2, space="PSUM"))

                    pend_norm = [None]

                    def attn_chunk(j):
                        nk = 4 * j + 4
                        for h in range(HPC):
                            ppv = ps_pv.tile([128, 512], dt.float32, tag="ppv", name="ppv")
                            pl = ps_l.tile([1, 512], dt.float32, tag="pl", name="pl")
                            post = [None]  # pl/ppv of tile ki-1, emitted
                            # after tile ki's scores so the PE never waits
                            # on the exp
                            for ki in range(nk):
                                ksl = slice(ki * 128, (ki + 1) * 128)
                                # diagonal tiles: columns left of the diagonal
                                # are fully masked -> compute only the suffix
                                off = max(0, (ki - 4 * j) * 128)
                                w = 512 - off
                                qs0 = j * 512 + off
                                ps = ps_s.tile([128, 512], dt.float32, tag="ps", name="ps")
                                nc.tensor.matmul(
                                    ps[:, off:], kt_n[h][:, ksl],
                                    qt_n[h][:, qs0:qs0 + w],
                                    start=True, stop=False, skip_group_check=True)
                                nc.tensor.matmul(
                                    ps[:, off:], kpe_t[64 * h:64 * h + 64, ksl],
                                    qt_r2[64 * h:64 * h + 64, qs0:qs0 + w],
                                    start=False, stop=True, skip_group_check=True)
                                if post[0] is not None:
                                    post[0]()
                                if pend_norm[0] is not None:
                                    pend_norm[0]()
                                    pend_norm[0] = None
                                if ki >= 4 * j:
                                    nc.vector.tensor_add(
                                        ps[:, off:], ps[:, off:], mask_sb[:, 0, :w])
                                pt = pt_pool.tile([128, 512], dt.bfloat16, tag="pt", name="pt")
                                nc.scalar.activation(pt[:, off:], ps[:, off:], AF.Exp)

                                def mk_post(ki=ki, off=off, pt=pt):
                                    nc.tensor.matmul(
                                        pl[:, off:], ones_b[:], pt[:, off:],
                                        start=(ki == 0), stop=(ki == nk - 1),
                                        skip_group_check=True)
                                    nc.tensor.matmul(
                                        ppv[:, off:],
                                        v2_t[ki][:, h * D_V:(h + 1) * D_V],
                                        pt[:, off:],
                                        start=(ki == 0), stop=(ki == nk - 1),
                                        skip_group_check=True)
                                post[0] = mk_post
                            post[0]()

                            def mk_norm(ppv=ppv, pl=pl, h=h):
                                # normalize: attn^T = ppv * (1/l) broadcast
                                rl = sm_pool.tile([1, 512], dt.float32, tag="rl", name="rl")
                                nc.vector.reciprocal_approx_fast(rl[:], pl[:])
                                rl16 = sm_pool.tile([1, 512], dt.float16, tag="rl16", name="rl16")
                                nc.scalar.copy(rl16[:], rl[:])
                                pb = ps_b.tile([128, 512], dt.float32, tag="pb", name="pb")
                                nc.tensor.matmul(pb[:], ones_f[:], rl16[:],
                                                 start=True, stop=True,
                                                 skip_group_check=True)
                                rb = sm_pool.tile([128, 512], dt.float32, tag="rb", name="rb")
                                nc.vector.tensor_copy(rb[:], pb[:])
                                attn = sm_pool.tile([128, 512], dt.bfloat16, tag="attn", name="attn")
                                nc.vector.tensor_mul(attn[:], ppv[:], rb[:])
                                nc.sync.dma_start(
                                    ag2_in[j][h * D_V:(h + 1) * D_V, :], attn[:])

                            if h == 0:
                                # defer head 0's normalize into head 1's
                                # first score tile
                                pend_norm[0] = mk_norm
                            else:
                                mk_norm()
                        nc.gpsimd.collective_compute(
                            "AllGather", mybir.AluOpType.bypass,
                            replica_groups=rg,
                            ins=[ag2_in[j].opt()], outs=[ag2_out[j].opt()])

                    def oproj_chunk(j):
                        jsl = slice(j * 512, (j + 1) * 512)
                        strips = [strip_pool.tile([128, 512], dt.bfloat16,
                                                  tag=f"st{kf}", name=f"st{kf}")
                                  for kf in range(16)]
                        for kf in range(16):
                            nc.sync.dma_start(
                                strips[kf][:],
                                ag2_out[j][kf * 128:(kf + 1) * 128, :])
                        for mt in range(7):
                            msl = slice(mt * 128, (mt + 1) * 128)
                            po = ps_e.tile([128, 512], dt.float32, tag="po", name="po")
                            for kf in range(16):
                                nc.tensor.matmul(
                                    po[:], wo_t[:, kf, msl], strips[kf][:],
                                    start=(kf == 0), stop=(kf == 15),
                                    skip_group_check=True)
                            ot = oo_pool.tile([128, 512], dt.float32, tag="ot", name="ot")
                            nc.scalar.copy(ot[:], po[:])
                            nc.sync.dma_start(out[msl, jsl], ot[:])

                    attn_chunk(0)
                    for j in range(1, 4):
                        attn_chunk(j)
                        oproj_chunk(j - 1)
                    oproj_chunk(3)


def _prep_inputs(hidden_states, positions, W_qkv_a, gamma_q, W_qb, gamma_kv,
                 W_kvb, W_o):
    f32 = np.float32
    perm = np.concatenate([np.arange(0, D_ROPE, 2), np.arange(1, D_ROPE, 2)])
    scale = np.float32(D_QK ** -0.5)

    # A-projection weights: de-interleave k_pe output cols, block layout
    Wa = np.asarray(W_qkv_a, f32).copy()
    Wa[:, QLR + KVLR:] = Wa[:, QLR + KVLR:][:, perm]
    Wa = np.concatenate([Wa, np.zeros((H, 64), f32)], axis=1)  # pad 2112->2176
    # chunk (m, kc) stored so each SBUF partition line is 2KB contiguous:
    # rows (m*56 + kc*8)*128 + p*8 + k8, cols f
    wa_b = (
        Wa.reshape(7, 8, 128, 17, 128)   # [kc, k8, p, m, f]
        .transpose(3, 0, 2, 1, 4)        # [m, kc, p, k8, f]
        .reshape(17 * 56 * 128, 128)
        .astype(BF16)
    )

    # q_b weights: fold gamma_q and score scale, de-interleave rope cols
    Wqb = (np.asarray(W_qb, f32) * np.asarray(gamma_q, f32)[:, None] * scale)
    Wqb = Wqb.reshape(QLR, NH, D_QK)
    Wqb = np.concatenate([Wqb[:, :, :D_NOPE], Wqb[:, :, D_NOPE:][:, :, perm]], axis=2)

    # kv_b weights: fold gamma_kv
    Wkvb = (np.asarray(W_kvb, f32) * np.asarray(gamma_kv, f32)[:, None])
    Wkvb = Wkvb.reshape(KVLR, NH, D_NOPE + D_V)

    Wo = np.asarray(W_o, f32)

    hTf = np.asarray(hidden_states, f32).T.astype(BF16)  # [H, T]

    pos = np.asarray(positions, f32)
    inv_freq = 1.0 / (THETA ** (np.arange(D_ROPE // 2, dtype=f32) / (D_ROPE // 2)))
    freqs = pos[:, None] * inv_freq[None, :]          # [T, 32]
    cos = np.cos(freqs).astype(f32).T                 # [32, T]
    sin = np.sin(freqs).astype(f32).T
    cs = np.concatenate([cos, sin], axis=0)           # [64, T]

    m = np.zeros((4, 128, 512), f32)
    kk = np.arange(128)[:, None]
    qq = np.arange(512)[None, :]
    for oi in range(4):
        m[oi][qq < kk + 128 * oi] = NEG
    masks = m.reshape(4 * 128, 512)

    in_maps = []
    for c in range(NCORES):
        hds = slice(2 * c, 2 * c + 2)
        in_maps.append({
            "hT": np.ascontiguousarray(hTf[:, c * TSH:(c + 1) * TSH]),
            "wa": wa_b,
            "wqb": np.ascontiguousarray(np.concatenate(
                [Wqb[:, 2 * c, :D_NOPE], Wqb[:, 2 * c + 1, :D_NOPE],
                 Wqb[:, 2 * c, D_NOPE:], Wqb[:, 2 * c + 1, D_NOPE:]],
                axis=1)).astype(BF16),
            "wkvb": np.ascontiguousarray(np.concatenate(
                [Wkvb[:, 2 * c, :D_NOPE], Wkvb[:, 2 * c + 1, :D_NOPE],
                 Wkvb[:, 2 * c, D_NOPE:], Wkvb[:, 2 * c + 1, D_NOPE:]],
                axis=1)).astype(BF16),
            "wo": np.ascontiguousarray(
                Wo[:, c * WO_COLS:(c + 1) * WO_COLS]).astype(BF16),
            "cs_sh": np.ascontiguousarray(cs[:, c * TSH:(c + 1) * TSH]),
            "cs_full": cs,
            "masks": masks,
        })
    return in_maps


def kernel(hidden_states, positions, W_qkv_a, gamma_q, W_qb, gamma_kv, W_kvb,
           W_o, _trace=False):
    from concourse.bass_utils import run_bass_kernel_spmd

    if "nc" not in _CACHE:
        _CACHE["nc"] = _build()
    nc = _CACHE["nc"]

    in_maps = _prep_inputs(hidden_states, positions, W_qkv_a, gamma_q, W_qb,
                           gamma_kv, W_kvb, W_o)
    res = run_bass_kernel_spmd(nc, in_maps, list(range(NCORES)), trace=_trace)
    _CACHE["last_result"] = res
    out = np.concatenate(
        [res.results[c]["out"].T for c in range(NCORES)], axis=1)
    return out.astype(np.float32)



# revision 25
# speedup vs baseline: 1.1759x; 1.0931x over previous
"""DeepseekV3 MLA prefill attention on 8 trn2 NeuronCores.

Strategy (single SPMD program, per-core differences live in the input data):
  Phase A: token-split A-projection, computed feature-major
           (qkv^T = W_a^T @ h^T), fused RMSNorm (partition-dim reduce via
           ones-matmul), RoPE on k_pe. gamma and the 1/sqrt(d) score scale
           are folded into the weights on the host; RoPE de-interleave is
           folded into weight column order on the host.
  AG1:     AllGather of normed latents (bf16, feature-major).
  Phase B: per-core head projections Q^T, K^T (feature-major) and V
           (token-major), heads 2c and 2c+1 on core c.
  Phase C: causal attention, S^T = K^T-tiles x Q^T-chunks, exp without
           max-subtraction (scores are O(+-8) by construction), softmax
           denominator via ones-matmul, PV accumulated feature-major,
           block-causal skipping of fully-masked tiles.
  AG2:     AllGather of attention outputs (bf16, feature-major).
  Phase E: column-split o_proj (core c computes output cols 896c..896(c+1)),
           host concatenates.
"""

import numpy as np
import ml_dtypes

T = 2048
H = 7168
NH = 16
D_NOPE = 128
D_ROPE = 64
D_V = 128
D_QK = 192
QLR = 1536
KVLR = 512
THETA = 10000.0
EPS = 1e-6
NCORES = 8
TSH = T // NCORES          # 256 tokens per core
HPC = NH // NCORES         # 2 heads per core
WO_COLS = H // NCORES      # 896 output cols per core
AGF = QLR + KVLR + D_ROPE  # 2112 gathered feature rows
NEG = -30000.0             # mask add, enough to zero bf16/f32 exp

BF16 = ml_dtypes.bfloat16

_CACHE = {}


class _Done(Exception):
    pass


def _build(upto='E'):
    import concourse.bass as bass
    import concourse.mybir as mybir
    import concourse.bacc as bacc
    import concourse.tile as tile

    dt = mybir.dt
    AF = mybir.ActivationFunctionType

    nc = bacc.Bacc(None, target_bir_lowering=False)

    # ---- per-core external inputs -------------------------------------
    hT = nc.declare_dram_parameter("hT", [H, TSH], dt.bfloat16, isOutput=False)
    wa = nc.declare_dram_parameter("wa", [17 * 56 * 128, 128], dt.bfloat16, isOutput=False)
    wqb = nc.declare_dram_parameter("wqb", [QLR, HPC * D_QK], dt.bfloat16, isOutput=False)
    wkvb = nc.declare_dram_parameter("wkvb", [KVLR, HPC * 256], dt.bfloat16, isOutput=False)
    wo = nc.declare_dram_parameter("wo", [NH * D_V, WO_COLS], dt.bfloat16, isOutput=False)
    cs_sh = nc.declare_dram_parameter("cs_sh", [64, TSH], dt.float32, isOutput=False)
    cs_full = nc.declare_dram_parameter("cs_full", [64, T], dt.float32, isOutput=False)
    masks = nc.declare_dram_parameter("masks", [4 * 128, 512], dt.float32, isOutput=False)
    out = nc.declare_dram_parameter("out", [WO_COLS, T], dt.float32, isOutput=True)

    rg = [list(range(NCORES))]

    _build_body(nc, mybir, upto, hT, wa, wqb, wkvb, wo, cs_sh, cs_full,
                masks, out)
    nc.compile()
    return nc


def _build_body(nc, mybir, upto, hT, wa, wqb, wkvb, wo, cs_sh, cs_full,
                masks, out):
    import concourse.tile as tile
    dt = mybir.dt
    AF = mybir.ActivationFunctionType
    rg = [list(range(NCORES))]

    with tile.TileContext(nc) as tc:
        import contextlib

        top = contextlib.ExitStack()
        with top:
            const = top.enter_context(tc.tile_pool(name="const", bufs=1))
            wpool = top.enter_context(tc.tile_pool(name="wpool", bufs=1))
            dram = top.enter_context(tc.tile_pool(name="dram", bufs=1, space="DRAM"))

            ones_b = const.tile([128, 1], dt.bfloat16, tag="ones_b", name="ones_b")
            nc.vector.memset(ones_b[:], 1.0)
            ones_f = const.tile([1, 128], dt.float16, tag="ones_f", name="ones_f")
            nc.vector.memset(ones_f[:], 1.0)
            # tiles allocated up front; DMAs for B/C-phase constants are
            # emitted at point of need so phase A's h/wa stream goes first
            mask_sb = const.tile([128, 4, 512], dt.float32, tag="mask", name="mask")
            # cos/sin replicated to every 32-partition block so DVE ops
            # stay partition-aligned for both heads (loads emitted at m==11)
            csc_r = const.tile([128, T], dt.float32, tag="csc_r", name="csc_r")
            csn_r = const.tile([128, T], dt.float32, tag="csn_r", name="csn_r")
            csc_s = const.tile([32, TSH], dt.float32, tag="csc_s", name="csc_s")
            nc.sync.dma_start(csc_s[:], cs_sh[0:32, :])
            csn_s = const.tile([32, TSH], dt.float32, tag="csn_s", name="csn_s")
            nc.sync.dma_start(csn_s[:], cs_sh[32:64, :])

            # resident weights for phases B and E (loads emitted later)
            wqb_t = wpool.tile([128, 12, HPC * D_QK], dt.bfloat16, tag="wqb", name="wqb")
            wkvb_t = wpool.tile([128, 4, HPC * 256], dt.bfloat16, tag="wkvb", name="wkvb")
            wo_t = wpool.tile([128, 16, WO_COLS], dt.bfloat16, tag="wo", name="wo")

            # collective buffers (AG1 split: kv+k_pe gathered early, q late;
            # AG2 split per head so it overlaps the other head's attention)
            ag1a_in = dram.tile([576, TSH], dt.bfloat16, tag="ag1ai", name="ag1ai")
            ag1a_out = dram.tile([NCORES * 576, TSH], dt.bfloat16, tag="ag1ao", name="ag1ao", addr_space="Shared")
            ag1b_in = dram.tile([QLR, TSH], dt.bfloat16, tag="ag1bi", name="ag1bi")
            ag1b_out = dram.tile([NCORES * QLR, TSH], dt.bfloat16, tag="ag1bo", name="ag1bo", addr_space="Shared")
            # AG2 split per 512-token chunk: chunk j's gather overlaps
            # attention on chunk j+1 and o_proj on chunk j-1
            ag2_in = [dram.tile([2 * D_V, 512], dt.bfloat16, tag=f"ag2i{j}",
                                name=f"ag2i{j}") for j in range(3)]
            ag2_out = [dram.tile([NCORES * 2 * D_V, 512], dt.bfloat16,
                                 tag=f"ag2o{j}", name=f"ag2o{j}",
                                 addr_space="Shared") for j in range(3)]
            # last chunk gathered per head so o_proj can start on head-0
            # strips while head 1 is still in flight
            ag3_in = [dram.tile([D_V, 512], dt.bfloat16, tag=f"ag3i{h}",
                                name=f"ag3i{h}") for h in range(HPC)]
            ag3_out = [dram.tile([NCORES * D_V, 512], dt.bfloat16,
                                 tag=f"ag3o{h}", name=f"ag3o{h}",
                                 addr_space="Shared") for h in range(HPC)]

            # ============================================================
            # Phase A: qkv^T = Wa^T @ h^T   [2112, 256] feature-major
            # ============================================================
            with contextlib.ExitStack() as pa:
                h_pool = pa.enter_context(tc.tile_pool(name="h", bufs=1))
                wa_pool = pa.enter_context(tc.tile_pool(name="wa", bufs=6))
                qkv_pool = pa.enter_context(tc.tile_pool(name="qkv", bufs=1))
                x2_pool = pa.enter_context(tc.tile_pool(name="x2", bufs=3))
                agt_pool = pa.enter_context(tc.tile_pool(name="agt", bufs=3))
                ps_a = pa.enter_context(tc.tile_pool(name="ps_a", bufs=3, space="PSUM"))
                ps_ss = pa.enter_context(tc.tile_pool(name="ps_ss", bufs=1, space="PSUM"))
                ps_bc = pa.enter_context(tc.tile_pool(name="ps_bc", bufs=1, space="PSUM"))

                h_all = h_pool.tile([128, 56, TSH], dt.bfloat16, tag="h_all", name="h_all")
                for a0 in range(0, 56, 7):
                    nc.sync.dma_start(
                        h_all[:, a0:a0 + 7, :],
                        hT[a0 * 128:(a0 + 7) * 128, :].rearrange(
                            "(a p) t -> p a t", p=128))

                qkv = [
                    qkv_pool.tile([128, TSH], dt.float32, tag=f"qkv{m}", name=f"qkv{m}")
                    for m in range(16)
                ]
                kp_raw = qkv_pool.tile([64, TSH], dt.float32, tag="kp_raw", name="kp_raw")
                kp2 = qkv_pool.tile([32, TSH], dt.float32, tag="kp2", name="kp2")

                ss_q = ps_ss.tile([1, TSH], dt.float32, tag="ssq", name="ssq")
                ss_kv = ps_ss.tile([1, TSH], dt.float32, tag="sskv", name="sskv")

                def rstd_bcast(ss, d, name):
                    ms = x2_pool.tile([1, TSH], dt.float32, tag="ms", name="ms")
                    nc.scalar.activation(ms[:], ss[:], AF.Copy, bias=EPS, scale=1.0 / d)
                    inv = x2_pool.tile([1, TSH], dt.float32, tag="inv", name="inv")
                    nc.vector.reciprocal_approx_fast(inv[:], ms[:])
                    rstd = x2_pool.tile([1, TSH], dt.float16, tag="rstd", name="rstd")
                    nc.scalar.activation(rstd[:], inv[:], AF.Sqrt)
                    pb = ps_bc.tile([128, TSH], dt.float32, tag=f"bc{name}", name=f"bc{name}")
                    nc.tensor.matmul(pb[:], ones_f[:], rstd[:], start=True, stop=True)
                    return pb

                pending = [None]

                def emit_ss(m):
                    # squared tile for the RMS partition-sum; deferred into
                    # the next group's matmul stream so the PE never waits
                    # on the Act-eviction -> DVE-square chain
                    x2 = x2_pool.tile([128, TSH], dt.bfloat16, tag="x2", name="x2")
                    nc.vector.tensor_mul(x2[:], qkv[m][:], qkv[m][:])
                    ss = ss_q if m < 12 else ss_kv
                    first = (m == 0) or (m == 12)
                    last = (m == 11) or (m == 15)
                    nc.tensor.matmul(
                        ss[:], ones_b[:], x2[:], start=first, stop=last,
                        skip_group_check=True,
                    )

                for m in list(range(12)) + [12, 13, 14, 15, 16]:
                    mp = 64 if m == 16 else 128
                    psum = ps_a.tile([128, TSH], dt.float32, tag="pa", name="pa")
                    for kc in range(7):
                        chunk = wa_pool.tile([128, 8, 128], dt.bfloat16, tag="wa_c", name="wa_c")
                        r0 = (m * 56 + kc * 8) * 128
                        nc.sync.dma_start(
                            chunk[:],
                            wa[r0:r0 + 1024, :].rearrange("(p a) f -> p a f", a=8),
                        )
                        for k8 in range(8):
                            k = kc * 8 + k8
                            nc.tensor.matmul(
                                psum[:mp, :],
                                chunk[:, k8, :mp],
                                h_all[:, k, :],
                                start=(k == 0),
                                stop=(k == 55),
                                skip_group_check=True,
                            )
                        if kc == 0 and pending[0] is not None:
                            pending[0]()
                            pending[0] = None
                    # evict to f32 SBUF
                    if m < 16:
                        nc.scalar.copy(qkv[m][:], psum[:])
                        if m == 11:
                            emit_ss(m)  # norm needs the full ss_q now
                        else:
                            pending[0] = (lambda m=m: emit_ss(m))
                    else:
                        nc.scalar.copy(kp_raw[:], psum[:64, :])
                        # move the x2 half to base partition 0 for the DVE ops
                        nc.sync.dma_start(kp2[:], kp_raw[32:64, :])
                        if pending[0] is not None:
                            pending[0]()
                            pending[0] = None
                    if m == 11:
                        # q group done: norm q, launch AG1b early so the
                        # gather overlaps the kv-group matmuls
                        bc_q = rstd_bcast(ss_q, QLR, "q")
                        for mm in range(12):
                            agt = agt_pool.tile([128, TSH], dt.bfloat16, tag="agt", name="agt")
                            nc.vector.tensor_mul(agt[:], qkv[mm][:], bc_q[:])
                            nc.sync.dma_start(
                                ag1b_in[mm * 128:(mm + 1) * 128, :], agt[:])
                        nc.gpsimd.collective_compute(
                            "AllGather", mybir.AluOpType.bypass,
                            replica_groups=rg,
                            ins=[ag1b_in.opt()], outs=[ag1b_out.opt()])
                    if m == 16:
                        # kv group + k_pe done: norm kv, rope k_pe, launch AG1a
                        bc_kv = rstd_bcast(ss_kv, KVLR, "kv")
                        for mm in range(12, 16):
                            agt = agt_pool.tile([128, TSH], dt.bfloat16, tag="agt", name="agt")
                            nc.vector.tensor_mul(agt[:], qkv[mm][:], bc_kv[:])
                            nc.sync.dma_start(
                                ag1a_in[(mm - 12) * 128:(mm - 11) * 128, :], agt[:])
                        kr1 = agt_pool.tile([32, TSH], dt.bfloat16, tag="kr1", name="kr1")
                        kr2 = agt_pool.tile([32, TSH], dt.bfloat16, tag="kr2", name="kr2")
                        t1 = x2_pool.tile([32, TSH], dt.bfloat16, tag="t1", name="t1")
                        t2 = x2_pool.tile([32, TSH], dt.bfloat16, tag="t2", name="t2")
                        nc.vector.tensor_mul(t1[:], kp_raw[0:32, :], csc_s[:])
                        nc.vector.tensor_mul(t2[:], kp2[:], csn_s[:])
                        nc.vector.tensor_sub(kr1[:], t1[:], t2[:])
                        t3 = x2_pool.tile([32, TSH], dt.bfloat16, tag="t1", name="t1")
                        t4 = x2_pool.tile([32, TSH], dt.bfloat16, tag="t2", name="t2")
                        nc.vector.tensor_mul(t3[:], kp_raw[0:32, :], csn_s[:])
                        nc.vector.tensor_mul(t4[:], kp2[:], csc_s[:])
                        nc.vector.tensor_add(kr2[:], t3[:], t4[:])
                        nc.sync.dma_start(ag1a_in[512:544, :], kr1[:])
                        nc.sync.dma_start(ag1a_in[544:576, :], kr2[:])
                        nc.gpsimd.collective_compute(
                            "AllGather", mybir.AluOpType.bypass,
                            replica_groups=rg,
                            ins=[ag1a_in.opt()], outs=[ag1a_out.opt()])
                        # B/C-phase weights + constants: emitted after the
                        # last wa chunk so they never starve phase A's stream
                        for a0 in range(0, 12, 4):
                            nc.sync.dma_start(
                                wqb_t[:, a0:a0 + 4, :],
                                wqb[a0 * 128:(a0 + 4) * 128, :].rearrange(
                                    "(a p) f -> p a f", p=128))
                        nc.sync.dma_start(
                            wkvb_t[:], wkvb.rearrange("(a p) f -> p a f", p=128))
                        for i in range(4):
                            nc.sync.dma_start(
                                mask_sb[:, i, :], masks[i * 128:(i + 1) * 128, :])
                        for b4 in range(4):
                            nc.sync.dma_start(
                                csc_r[b4 * 32:(b4 + 1) * 32, :], cs_full[0:32, :])
                            nc.sync.dma_start(
                                csn_r[b4 * 32:(b4 + 1) * 32, :], cs_full[32:64, :])

            if upto == 'A':
                return

            # ============================================================
            # Phase B: Q^T, K^T (feature-major) and V (token-major)
            # ============================================================
            bpools = contextlib.ExitStack()
            with bpools:
                act = bpools.enter_context(tc.tile_pool(name="act", bufs=1))
                agq_pool = bpools.enter_context(tc.tile_pool(name="agq", bufs=6))
                agkv_pool = bpools.enter_context(tc.tile_pool(name="agkv", bufs=2))
                pbp = contextlib.ExitStack()
                ps_q = pbp.enter_context(tc.tile_pool(name="ps_q", bufs=6, space="PSUM"))

                qt_n = [act.tile([128, T], dt.bfloat16, tag=f"qtn{h}", name=f"qtn{h}") for h in range(HPC)]
                # merged rope layout: partitions [64h, 64h+32) = head h x1,
                # [64h+32, 64h+64) = head h x2
                qrw2 = act.tile([128, T], dt.bfloat16, tag="qrw2", name="qrw2")
                qt_r2 = act.tile([128, T], dt.bfloat16, tag="qtr2", name="qtr2")
                kt_n = [act.tile([128, T], dt.bfloat16, tag=f"ktn{h}", name=f"ktn{h}") for h in range(HPC)]
                # k_pe replicated on both 64-partition halves so each
                # head's rope matmul is partition-aligned with qt_r2
                kpe_t = act.tile([128, T], dt.bfloat16, tag="kpet", name="kpet")
                # v2_t[ti]: both heads' V for token tile ti, cols h*128..
                v2_t = [act.tile([128, 2 * D_V], dt.bfloat16, tag=f"v{i}", name=f"v{i}")
                        for i in range(16)]

                for rp in range(4):
                    tsl = slice(rp * 512, (rp + 1) * 512)
                    # --- Q path (two ranks per 512-wide chunk) ---
                    pn0 = ps_q.tile([128, 512], dt.float32, tag="pq", name="pq")
                    pn1 = ps_q.tile([128, 512], dt.float32, tag="pq", name="pq")
                    pr2 = ps_q.tile([128, 512], dt.float32, tag="pq", name="pq")
                    for kq in range(12):
                        aq = agq_pool.tile([128, 512], dt.bfloat16, tag="aq", name="aq")
                        for s in range(2):
                            r = 2 * rp + s
                            nc.sync.dma_start(
                                aq[:, s * TSH:(s + 1) * TSH],
                                ag1b_out[r * QLR + kq * 128: r * QLR + (kq + 1) * 128, :])
                        nc.tensor.matmul(
                            pn0[:], wqb_t[:, kq, 0:128], aq[:],
                            start=(kq == 0), stop=(kq == 11),
                            skip_group_check=True)
                        nc.tensor.matmul(
                            pn1[:], wqb_t[:, kq, 128:256], aq[:],
                            start=(kq == 0), stop=(kq == 11),
                            skip_group_check=True)
                        nc.tensor.matmul(
                            pr2[:], wqb_t[:, kq, 256:384], aq[:],
                            start=(kq == 0), stop=(kq == 11),
                            skip_group_check=True)
                    nc.scalar.copy(qt_n[0][:, tsl], pn0[:])
                    nc.scalar.copy(qt_n[1][:, tsl], pn1[:])
                    nc.scalar.copy(qrw2[:, tsl], pr2[:])
                    # --- RoPE on q for this 512-token chunk; per head all
                    # DVE ops live on partitions [64h, 64h+32) ---
                    for h in range(HPC):
                        p0 = 64 * h
                        x2c = agq_pool.tile([128, 512], dt.bfloat16, tag="x2c", name="x2c")
                        nc.sync.dma_start(
                            x2c[p0:p0 + 32, :], qrw2[p0 + 32:p0 + 64, tsl])
                        x1 = qrw2[p0:p0 + 32, tsl]
                        x2 = x2c[p0:p0 + 32, :]
                        cs_ = csc_r[p0:p0 + 32, tsl]
                        sn_ = csn_r[p0:p0 + 32, tsl]
                        ta = agq_pool.tile([128, 512], dt.bfloat16, tag="qt1", name="qt1")
                        tb = agq_pool.tile([128, 512], dt.bfloat16, tag="qt2", name="qt2")
                        nc.vector.tensor_mul(ta[p0:p0 + 32, :], x1, cs_)
                        nc.vector.tensor_mul(tb[p0:p0 + 32, :], x2, sn_)
                        nc.vector.tensor_sub(
                            qt_r2[p0:p0 + 32, tsl], ta[p0:p0 + 32, :], tb[p0:p0 + 32, :])
                        tg = agq_pool.tile([128, 512], dt.bfloat16, tag="qt1", name="qt1")
                        td = agq_pool.tile([128, 512], dt.bfloat16, tag="qt2", name="qt2")
                        nc.vector.tensor_mul(tg[p0:p0 + 32, :], x1, sn_)
                        nc.vector.tensor_mul(td[p0:p0 + 32, :], x2, cs_)
                        r2t = agq_pool.tile([128, 512], dt.bfloat16, tag="r2t", name="r2t")
                        nc.vector.tensor_add(
                            r2t[p0:p0 + 32, :], tg[p0:p0 + 32, :], td[p0:p0 + 32, :])
                        nc.sync.dma_start(qt_r2[p0 + 32:p0 + 64, tsl], r2t[p0:p0 + 32, :])

                pbp.close()
                pbp = contextlib.ExitStack()
                ps_kv = pbp.enter_context(tc.tile_pool(name="ps_kv", bufs=2, space="PSUM"))

                # KV/V second: AG1a lands after AG1b on the CC queue,
                # so the Q work above covers its transfer
                for rp in range(4):
                    tsl = slice(rp * 512, (rp + 1) * 512)
                    # --- KV path ---
                    akv = [agkv_pool.tile([128, 512], dt.bfloat16, tag=f"akv{kk}", name=f"akv{kk}")
                           for kk in range(4)]
                    for kk in range(4):
                        for s in range(2):
                            r = 2 * rp + s
                            nc.sync.dma_start(
                                akv[kk][:, s * TSH:(s + 1) * TSH],
                                ag1a_out[r * 576 + kk * 128: r * 576 + (kk + 1) * 128, :])
                    for h in range(HPC):
                        pk = ps_kv.tile([128, 512], dt.float32, tag="pkv", name="pkv")
                        for kk in range(4):
                            nc.tensor.matmul(
                                pk[:], wkvb_t[:, kk, h * 128:(h + 1) * 128], akv[kk][:],
                                start=(kk == 0), stop=(kk == 3),
                                skip_group_check=True)
                        nc.scalar.copy(kt_n[h][:, tsl], pk[:])
                    # --- V for both heads at once (wkvb cols 256..512) ---
                    for s4 in range(4):
                        ti = 4 * rp + s4
                        pv = ps_kv.tile([128, 2 * D_V], dt.float32, tag="pv_b", name="pv_b")
                        for kk in range(4):
                            nc.tensor.matmul(
                                pv[:],
                                akv[kk][:, s4 * 128:(s4 + 1) * 128],
                                wkvb_t[:, kk, 256:512],
                                start=(kk == 0), stop=(kk == 3),
                                skip_group_check=True)
                        nc.scalar.copy(v2_t[ti][:], pv[:])
                    # --- shared roped k_pe (both halves) ---
                    for s in range(2):
                        r = 2 * rp + s
                        for hh in range(2):
                            nc.sync.dma_start(
                                kpe_t[64 * hh:64 * hh + 64, r * TSH:(r + 1) * TSH],
                                ag1a_out[r * 576 + 512: r * 576 + 576, :])

                pbp.close()

                # E weights: load during B/C
                for a0 in range(0, 16, 4):
                    nc.sync.dma_start(
                        wo_t[:, a0:a0 + 4, :],
                        wo[a0 * 128:(a0 + 4) * 128, :].rearrange(
                            "(a p) f -> p a f", p=128))

                if upto == 'B':
                    return
                # ========================================================
                # Phase C+E: attention (j-outer), per-chunk AllGather, and
                # o_proj of chunk j-1 overlapped with attention on chunk j
                # ========================================================
                with contextlib.ExitStack() as pc:
                    pt_pool = pc.enter_context(tc.tile_pool(name="pt", bufs=3))
                    sm_pool = pc.enter_context(tc.tile_pool(name="sm", bufs=2))
                    strip_pool = pc.enter_context(tc.tile_pool(name="strips", bufs=2))
                    oo_pool = pc.enter_context(tc.tile_pool(name="oo", bufs=3))
                    ps_s = pc.enter_context(tc.tile_pool(name="ps_s", bufs=2, space="PSUM"))
                    ps_pv = pc.enter_context(tc.tile_pool(name="ps_pv", bufs=2, space="PSUM"))
                    ps_l = pc.enter_context(tc.tile_pool(name="ps_l", bufs=1, space="PSUM"))
                    ps_b = pc.enter_context(tc.tile_pool(name="ps_b", bufs=1, space="PSUM"))
                    ps_e = pc.enter_context(tc.tile_pool(name="ps_e", bufs=2, space="PSUM"))

                    pend_norm = [None]

                    def attn_chunk(j):
                        nk = 4 * j + 4
                        for h in range(HPC):
                            ppv = ps_pv.tile([128, 512], dt.float32, tag="ppv", name="ppv")
                            pl = ps_l.tile([1, 512], dt.float32, tag="pl", name="pl")
                            post = [None]  # pl/ppv of tile ki-1, emitted
                            # after tile ki's scores so the PE never waits
                            # on the exp
                            for ki in range(nk):
                                ksl = slice(ki * 128, (ki + 1) * 128)
                                # diagonal tiles: columns left of the diagonal
                                # are fully masked -> compute only the suffix
                                off = max(0, (ki - 4 * j) * 128)
                                w = 512 - off
                                qs0 = j * 512 + off
                                ps = ps_s.tile([128, 512], dt.float32, tag="ps", name="ps")
                                nc.tensor.matmul(
                                    ps[:, off:], kt_n[h][:, ksl],
                                    qt_n[h][:, qs0:qs0 + w],
                                    start=True, stop=False, skip_group_check=True)
                                nc.tensor.matmul(
                                    ps[:, off:], kpe_t[64 * h:64 * h + 64, ksl],
                                    qt_r2[64 * h:64 * h + 64, qs0:qs0 + w],
                                    start=False, stop=True, skip_group_check=True)
                                if post[0] is not None:
                                    post[0]()
                                if pend_norm[0] is not None:
                                    pend_norm[0]()
                                    pend_norm[0] = None
                                if ki >= 4 * j:
                                    nc.vector.tensor_add(
                                        ps[:, off:], ps[:, off:], mask_sb[:, 0, :w])
                                pt = pt_pool.tile([128, 512], dt.bfloat16, tag="pt", name="pt")
                                nc.scalar.activation(pt[:, off:], ps[:, off:], AF.Exp)

                                def mk_post(ki=ki, off=off, pt=pt):
                                    nc.tensor.matmul(
                                        pl[:, off:], ones_b[:], pt[:, off:],
                                        start=(ki == 0), stop=(ki == nk - 1),
                                        skip_group_check=True)
                                    nc.tensor.matmul(
                                        ppv[:, off:],
                                        v2_t[ki][:, h * D_V:(h + 1) * D_V],
                                        pt[:, off:],
                                        start=(ki == 0), stop=(ki == nk - 1),
                                        skip_group_check=True)
                                post[0] = mk_post
                            post[0]()

                            def mk_norm(ppv=ppv, pl=pl, h=h):
                                # normalize: attn^T = ppv * (1/l) broadcast
                                rl = sm_pool.tile([1, 512], dt.float32, tag="rl", name="rl")
                                nc.vector.reciprocal_approx_fast(rl[:], pl[:])
                                rl16 = sm_pool.tile([1, 512], dt.float16, tag="rl16", name="rl16")
                                nc.scalar.copy(rl16[:], rl[:])
                                pb = ps_b.tile([128, 512], dt.float32, tag="pb", name="pb")
                                nc.tensor.matmul(pb[:], ones_f[:], rl16[:],
                                                 start=True, stop=True,
                                                 skip_group_check=True)
                                rb = sm_pool.tile([128, 512], dt.float32, tag="rb", name="rb")
                                nc.vector.tensor_copy(rb[:], pb[:])
                                attn = sm_pool.tile([128, 512], dt.bfloat16, tag="attn", name="attn")
                                nc.vector.tensor_mul(attn[:], ppv[:], rb[:])
                                if j == 3:
                                    nc.sync.dma_start(ag3_in[h][:], attn[:])
                                else:
                                    nc.sync.dma_start(
                                        ag2_in[j][h * D_V:(h + 1) * D_V, :], attn[:])

                            if j == 3:
                                mk_norm()
                                nc.gpsimd.collective_compute(
                                    "AllGather", mybir.AluOpType.bypass,
                                    replica_groups=rg,
                                    ins=[ag3_in[h].opt()],
                                    outs=[ag3_out[h].opt()])
                            elif h == 0:
                                # defer head 0's normalize into head 1's
                                # first score tile
                                pend_norm[0] = mk_norm
                            else:
                                mk_norm()
                        if j < 3:
                            nc.gpsimd.collective_compute(
                                "AllGather", mybir.AluOpType.bypass,
                                replica_groups=rg,
                                ins=[ag2_in[j].opt()], outs=[ag2_out[j].opt()])

                    def oproj_chunk(j):
                        jsl = slice(j * 512, (j + 1) * 512)
                        strips = [strip_pool.tile([128, 512], dt.bfloat16,
                                                  tag=f"st{kf}", name=f"st{kf}")
                                  for kf in range(16)]
                        if j == 3:
                            # even heads (gathered first) accumulated first
                            kf_order = [2 * r for r in range(8)] + \
                                       [2 * r + 1 for r in range(8)]
                            for kf in kf_order:
                                nc.sync.dma_start(
                                    strips[kf][:],
                                    ag3_out[kf % 2][(kf // 2) * 128:
                                                    (kf // 2 + 1) * 128, :])
                        else:
                            kf_order = list(range(16))
                            for kf in kf_order:
                                nc.sync.dma_start(
                                    strips[kf][:],
                                    ag2_out[j][kf * 128:(kf + 1) * 128, :])
                        for mt in range(7):
                            msl = slice(mt * 128, (mt + 1) * 128)
                            po = ps_e.tile([128, 512], dt.float32, tag="po", name="po")
                            for i, kf in enumerate(kf_order):
                                nc.tensor.matmul(
                                    po[:], wo_t[:, kf, msl], strips[kf][:],
                                    start=(i == 0), stop=(i == 15),
                                    skip_group_check=True)
                            ot = oo_pool.tile([128, 512], dt.float32, tag="ot", name="ot")
                            nc.scalar.copy(ot[:], po[:])
                            nc.sync.dma_start(out[msl, jsl], ot[:])

                    attn_chunk(0)
                    for j in range(1, 4):
                        attn_chunk(j)
                        oproj_chunk(j - 1)
                    oproj_chunk(3)


def _prep_inputs(hidden_states, positions, W_qkv_a, gamma_q, W_qb, gamma_kv,
                 W_kvb, W_o):
    f32 = np.float32
    perm = np.concatenate([np.arange(0, D_ROPE, 2), np.arange(1, D_ROPE, 2)])
    scale = np.float32(D_QK ** -0.5)

    # A-projection weights: de-interleave k_pe output cols, block layout
    Wa = np.asarray(W_qkv_a, f32).copy()
    Wa[:, QLR + KVLR:] = Wa[:, QLR + KVLR:][:, perm]
    Wa = np.concatenate([Wa, np.zeros((H, 64), f32)], axis=1)  # pad 2112->2176
    # chunk (m, kc) stored so each SBUF partition line is 2KB contiguous:
    # rows (m*56 + kc*8)*128 + p*8 + k8, cols f
    wa_b = (
        Wa.reshape(7, 8, 128, 17, 128)   # [kc, k8, p, m, f]
        .transpose(3, 0, 2, 1, 4)        # [m, kc, p, k8, f]
        .reshape(17 * 56 * 128, 128)
        .astype(BF16)
    )

    # q_b weights: fold gamma_q and score scale, de-interleave rope cols
    Wqb = (np.asarray(W_qb, f32) * np.asarray(gamma_q, f32)[:, None] * scale)
    Wqb = Wqb.reshape(QLR, NH, D_QK)
    Wqb = np.concatenate([Wqb[:, :, :D_NOPE], Wqb[:, :, D_NOPE:][:, :, perm]], axis=2)

    # kv_b weights: fold gamma_kv
    Wkvb = (np.asarray(W_kvb, f32) * np.asarray(gamma_kv, f32)[:, None])
    Wkvb = Wkvb.reshape(KVLR, NH, D_NOPE + D_V)

    Wo = np.asarray(W_o, f32)

    hTf = np.asarray(hidden_states, f32).T.astype(BF16)  # [H, T]

    pos = np.asarray(positions, f32)
    inv_freq = 1.0 / (THETA ** (np.arange(D_ROPE // 2, dtype=f32) / (D_ROPE // 2)))
    freqs = pos[:, None] * inv_freq[None, :]          # [T, 32]
    cos = np.cos(freqs).astype(f32).T                 # [32, T]
    sin = np.sin(freqs).astype(f32).T
    cs = np.concatenate([cos, sin], axis=0)           # [64, T]

    m = np.zeros((4, 128, 512), f32)
    kk = np.arange(128)[:, None]
    qq = np.arange(512)[None, :]
    for oi in range(4):
        m[oi][qq < kk + 128 * oi] = NEG
    masks = m.reshape(4 * 128, 512)

    in_maps = []
    for c in range(NCORES):
        hds = slice(2 * c, 2 * c + 2)
        in_maps.append({
            "hT": np.ascontiguousarray(hTf[:, c * TSH:(c + 1) * TSH]),
            "wa": wa_b,
            "wqb": np.ascontiguousarray(np.concatenate(
                [Wqb[:, 2 * c, :D_NOPE], Wqb[:, 2 * c + 1, :D_NOPE],
                 Wqb[:, 2 * c, D_NOPE:], Wqb[:, 2 * c + 1, D_NOPE:]],
                axis=1)).astype(BF16),
            "wkvb": np.ascontiguousarray(np.concatenate(
                [Wkvb[:, 2 * c, :D_NOPE], Wkvb[:, 2 * c + 1, :D_NOPE],
                 Wkvb[:, 2 * c, D_NOPE:], Wkvb[:, 2 * c + 1, D_NOPE:]],
                axis=1)).astype(BF16),
            "wo": np.ascontiguousarray(
                Wo[:, c * WO_COLS:(c + 1) * WO_COLS]).astype(BF16),
            "cs_sh": np.ascontiguousarray(cs[:, c * TSH:(c + 1) * TSH]),
            "cs_full": cs,
            "masks": masks,
        })
    return in_maps


def kernel(hidden_states, positions, W_qkv_a, gamma_q, W_qb, gamma_kv, W_kvb,
           W_o, _trace=False):
    from concourse.bass_utils import run_bass_kernel_spmd

    if "nc" not in _CACHE:
        _CACHE["nc"] = _build()
    nc = _CACHE["nc"]

    in_maps = _prep_inputs(hidden_states, positions, W_qkv_a, gamma_q, W_qb,
                           gamma_kv, W_kvb, W_o)
    res = run_bass_kernel_spmd(nc, in_maps, list(range(NCORES)), trace=_trace)
    _CACHE["last_result"] = res
    out = np.concatenate(
        [res.results[c]["out"].T for c in range(NCORES)], axis=1)
    return out.astype(np.float32)

